# revision 8
# baseline (speedup 1.0000x reference)
"""4-layer transformer encoder (B=2, S=2048, D=1024, FF=4096, H=16) on 8 TRN2
NeuronCores.

Sharding: 4096 tokens split 512/core (cores 0-3 = batch 0, 4-7 = batch 1).
Weights replicated. Per layer: local QKV projections in fp32r, AllGather of
K^T/V (bf16) within each 4-core quad, attention computed as
E^T = exp(K.Q^T/sqrt(dh)) with softmax denominator from a ones-augmented V
matmul, output projection + residual + LayerNorm (partition-dim stats via
ones-matmuls), then FFN + residual + LayerNorm.

Activations live transposed in SBUF (x^T: [D partitions, tokens free]) so no
on-device transposes are needed anywhere; the host transposes the input shard
once and the output shard back.
"""
import sys
if '/opt/trn_rl_repo' not in sys.path:
    sys.path.insert(0, '/opt/trn_rl_repo')

import numpy as np

import concourse.bass as bass
import concourse.mybir as mybir
import concourse.tile as tile
import concourse.bacc as bacc
from concourse import bass_utils

# problem config (hardcoded per contest rules)
L = 4
D = 1024
FF = 4096
H = 16
DH = 64
B = 2
S = 2048
EPS = 1e-6
SCALE = 1.0 / 8.0  # 1/sqrt(DH)

NCORES = 8
TOK = 512           # tokens per core
P = 128
DC = D // P         # 8 d-chunks
FC = FF // P        # 32 ff-chunks
NK = S // P         # 16 k-token chunks
R = 4               # ranks per quad (cores sharing one batch element)
RGROUPS = [[0, 1, 2, 3], [4, 5, 6, 7]]

KSZ = D * TOK       # elements in one core's K^T contribution
VSZ = TOK * D       # elements in one core's V contribution

dt = mybir.dt
AF = mybir.ActivationFunctionType
OP = mybir.AluOpType


def build(n_layers=L):
    nc = bacc.Bacc("TRN2", target_bir_lowering=False, debug=False,
                   num_devices=NCORES)
    f32, f32r, bf16 = dt.float32, dt.float32r, dt.bfloat16

    xT_d = nc.dram_tensor("xT", [D, TOK], f32, kind="ExternalInput")
    wq_d = nc.dram_tensor("wq", [n_layers, D, D], f32, kind="ExternalInput")
    wk_d = nc.dram_tensor("wk", [n_layers, D, D], f32, kind="ExternalInput")
    wv_d = nc.dram_tensor("wv", [n_layers, D, D], f32, kind="ExternalInput")
    wp_d = nc.dram_tensor("wp", [n_layers, D, D], f32, kind="ExternalInput")
    w1_d = nc.dram_tensor("w1", [n_layers, D, FF], f32, kind="ExternalInput")
    w2_d = nc.dram_tensor("w2", [n_layers, FF, D], f32, kind="ExternalInput")
    bq_d = nc.dram_tensor("bq", [n_layers, D], f32, kind="ExternalInput")
    bk_d = nc.dram_tensor("bk", [n_layers, D], f32, kind="ExternalInput")
    bp_d = nc.dram_tensor("bp", [n_layers, D], f32, kind="ExternalInput")
    b1_d = nc.dram_tensor("b1", [n_layers, FF], f32, kind="ExternalInput")
    b2_d = nc.dram_tensor("b2", [n_layers, D], f32, kind="ExternalInput")
    g1_d = nc.dram_tensor("g1", [n_layers, D], f32, kind="ExternalInput")
    be1_d = nc.dram_tensor("be1", [n_layers, D], f32, kind="ExternalInput")
    g2_d = nc.dram_tensor("g2", [n_layers, D], f32, kind="ExternalInput")
    be2_d = nc.dram_tensor("be2", [n_layers, D], f32, kind="ExternalInput")
    out_d = nc.dram_tensor("outT", [D, TOK], f32, kind="ExternalOutput")

    with tile.TileContext(nc) as tc:
        with (
            tc.tile_pool(name="pers", bufs=1) as pers,
            tc.tile_pool(name="sb", bufs=1) as sb,
            tc.tile_pool(name="ps", bufs=1, space="PSUM") as ps,
            tc.tile_pool(name="dram", bufs=1, space="DRAM") as dram,
        ):
            ones_f = pers.tile([P, P], f32)
            nc.vector.memset(ones_f[:], 1.0)
            ones = pers.tile([P, P], f32r)
            nc.vector.tensor_copy(out=ones[:], in_=ones_f[:])
            eps_sb = pers.tile([1, 1], f32)
            nc.vector.memset(eps_sb[:], EPS)

            def load_param(name, src, nchunk):
                t = pers.tile([P, n_layers, nchunk], f32, name=name)
                nc.sync.dma_start(
                    t[:], src[:, :].rearrange("l (c p) -> p l c", p=P))
                return t

            bq_sb = load_param("bq_sb", bq_d, DC)
            bk_sb = load_param("bk_sb", bk_d, DC)
            bp_sb = load_param("bp_sb", bp_d, DC)
            b2_sb = load_param("b2_sb", b2_d, DC)
            g1_sb = load_param("g1_sb", g1_d, DC)
            be1_sb = load_param("be1_sb", be1_d, DC)
            g2_sb = load_param("g2_sb", g2_d, DC)
            be2_sb = load_param("be2_sb", be2_d, DC)
            b1_sb = load_param("b1_sb", b1_d, FC)

            xT = sb.tile([P, DC, TOK], f32r, tag="xT", bufs=2, name="xT0")
            nc.sync.dma_start(
                xT[:],
                xT_d[:, :].rearrange("(c p) t -> p c t", p=P).bitcast(f32r))

            def wtile(src_ap, name):
                """Stream a [D-contraction, 256-out-cols] weight block into
                SBUF as [P, DC, 256] fp32r. src_ap: [kdim, 256] DRAM slice."""
                t = sb.tile([P, src_ap.shape[0] // P, 256], f32r,
                            tag="wblk", bufs=2, name=name)
                nc.sync.dma_start(
                    t[:],
                    src_ap.rearrange("(kc p) o -> p kc o", p=P).bitcast(f32r))
                return t

            def layernorm(l, t1, g_sb, be_sb, xout):
                """xout[:, c, :] = LN(t1) over the partition (d) axis."""
                psum_s = ps.tile([1, TOK], f32, tag="stat", bufs=2,
                                 name="psum_s")
                psum_sq = ps.tile([1, TOK], f32, tag="stat", bufs=2,
                                  name="psum_sq")
                for c in range(DC):
                    nc.tensor.matmul(psum_s[:], ones[:, 0:1], t1[:, c, :],
                                     start=(c == 0), stop=(c == DC - 1))
                for c in range(DC):
                    sqc = sb.tile([P, TOK], f32r, tag="sq", bufs=2, name="sqc")
                    nc.scalar.square(sqc[:], t1[:, c, :])
                    nc.tensor.matmul(psum_sq[:], ones[:, 0:1], sqc[:],
                                     start=(c == 0), stop=(c == DC - 1))
                mean = sb.tile([1, TOK], f32r, tag="vec", bufs=4, name="mean")
                nc.vector.tensor_scalar_mul(mean[:], psum_s[:], 1.0 / D)
                ms = sb.tile([1, TOK], f32, tag="vec", bufs=4, name="ms")
                nc.vector.tensor_scalar_mul(ms[:], psum_sq[:], 1.0 / D)
                var = sb.tile([1, TOK], f32, tag="vec", bufs=4, name="var")
                # var = ms - mean*mean = (mean * -mean) + ms
                nc.vector.scalar_tensor_tensor(
                    out=var[:], in0=mean[:].bitcast(f32), scalar=-1.0,
                    in1=mean[:].bitcast(f32), op0=OP.mult, op1=OP.mult)
                nc.vector.tensor_sub(var[:], ms[:], var[:])
                std = sb.tile([1, TOK], f32, tag="vec", bufs=4, name="std")
                nc.scalar.activation(std[:], var[:], AF.Sqrt, bias=eps_sb[:])
                rstd = sb.tile([1, TOK], f32r, tag="vec", bufs=4, name="rstd")
                with nc.allow_low_precision("fp32r rstd for PE broadcast"):
                    nc.vector.reciprocal(rstd[:], std[:])
                pm = ps.tile([P, TOK], f32, tag="mm", bufs=4, name="pm")
                nc.tensor.matmul(pm[:], ones[0:1, :], mean[:],
                                 start=True, stop=True)
                mrep = sb.tile([P, TOK], f32, tag="mrep", bufs=1, name="mrep")
                nc.scalar.copy(mrep[:], pm[:])
                pr = ps.tile([P, TOK], f32, tag="mm", bufs=4, name="pr")
                nc.tensor.matmul(pr[:], ones[0:1, :], rstd[:],
                                 start=True, stop=True)
                rrep = sb.tile([P, TOK], f32, tag="rrepLN", bufs=1,
                               name="rrep")
                nc.scalar.copy(rrep[:], pr[:])
                for c in range(DC):
                    d1 = sb.tile([P, TOK], f32, tag="lnscr", bufs=3,
                                 name="d1")
                    nc.vector.tensor_sub(d1[:], t1[:, c, :].bitcast(f32),
                                         mrep[:])
                    d2 = sb.tile([P, TOK], f32, tag="lnscr", bufs=3,
                                 name="d2")
                    nc.vector.tensor_mul(d2[:], d1[:], rrep[:])
                    nc.vector.tensor_scalar(
                        out=xout[:, c, :], in0=d2[:],
                        scalar1=g_sb[:, l, c:c + 1],
                        scalar2=be_sb[:, l, c:c + 1],
                        op0=OP.mult, op1=OP.add)

            for l in range(n_layers):
                # ---------------- K projection (staged to gather input) ----
                ccK = dram.tile([DC, P, TOK], bf16, tag="ccK", bufs=2,
                                name="ccK")
                for g in range(4):
                    wt = wtile(wk_d[l, :, 256 * g:256 * (g + 1)], "wkt")
                    for cc in range(2):
                        c = 2 * g + cc
                        pk = ps.tile([P, TOK], f32, tag="mm", bufs=4,
                                     name="pk")
                        for kc in range(DC):
                            nc.tensor.matmul(
                                pk[:], wt[:, kc, 128 * cc:128 * (cc + 1)],
                                xT[:, kc, :],
                                start=(kc == 0), stop=(kc == DC - 1))
                        kst = sb.tile([P, TOK], bf16, tag="kvstage", bufs=2,
                                      name="kst")
                        nc.scalar.activation(kst[:], pk[:], AF.Identity,
                                             bias=bk_sb[:, l, c:c + 1])
                        nc.sync.dma_start(ccK[c, :, :].rearrange("p t -> p t"),
                                          kst[:])
                ccKo = dram.tile([R, DC, P, TOK], bf16, tag="ccKo", bufs=2,
                                 name="ccKo")
                nc.gpsimd.collective_compute(
                    "AllGather", OP.bypass, replica_groups=RGROUPS,
                    ins=[ccK[:].opt()], outs=[ccKo[:].opt()])

                # ---------------- V projection (token-major, staged) -------
                ccV = dram.tile([4, P, D], bf16, tag="ccV", bufs=2,
                                name="ccV")
                for g in range(4):
                    wt = wtile(wv_d[l, :, 256 * g:256 * (g + 1)], "wvt")
                    for tc in range(4):
                        pv = ps.tile([P, 256], f32, tag="mm", bufs=4,
                                     name="pv")
                        for kc in range(DC):
                            nc.tensor.matmul(
                                pv[:], xT[:, kc, 128 * tc:128 * (tc + 1)],
                                wt[:, kc, :],
                                start=(kc == 0), stop=(kc == DC - 1))
                        vst = sb.tile([P, 256], bf16, tag="vstage", bufs=2,
                                      name="vst")
                        nc.vector.tensor_copy(out=vst[:], in_=pv[:])
                        nc.sync.dma_start(
                            ccV[tc, :, 256 * g:256 * (g + 1)], vst[:])
                ccVo = dram.tile([R, 4, P, D], bf16, tag="ccVo", bufs=2,
                                 name="ccVo")
                nc.gpsimd.collective_compute(
                    "AllGather", OP.bypass, replica_groups=RGROUPS,
                    ins=[ccV[:].opt()], outs=[ccVo[:].opt()])

                # ---------------- Q projection (stays local, bf16) ---------
                QT = sb.tile([P, DC, TOK], bf16, tag="QT", bufs=1, name="QT")
                for g in range(4):
                    wt = wtile(wq_d[l, :, 256 * g:256 * (g + 1)], "wqt")
                    for cc in range(2):
                        c = 2 * g + cc
                        pq = ps.tile([P, TOK], f32, tag="mm", bufs=4,
                                     name="pq")
                        for kc in range(DC):
                            nc.tensor.matmul(
                                pq[:], wt[:, kc, 128 * cc:128 * (cc + 1)],
                                xT[:, kc, :],
                                start=(kc == 0), stop=(kc == DC - 1))
                        nc.scalar.activation(QT[:, c, :], pq[:], AF.Identity,
                                             bias=bq_sb[:, l, c:c + 1])

                # ---------------- gathered K/V into SBUF -------------------
                v_sb = sb.tile([P, R, 4, H, DH + 1], bf16, tag="Vg", bufs=1,
                               name="v_sb")
                nc.vector.memset(v_sb[:, :, :, :, DH:DH + 1], 1.0)
                for r_ in range(R):
                    for tc in range(4):
                        nc.sync.dma_start(
                            v_sb[:, r_, tc, :, 0:DH],
                            ccVo[r_, tc].rearrange("p (h d) -> p h d", d=DH))

                # ---------------- attention, head pair per c ---------------
                ctxT = sb.tile([P, DC, TOK], f32r, tag="big2", bufs=2,
                               name="ctxT")
                for c in range(DC):
                    ktc = sb.tile([P, R, TOK], bf16, tag="KTc", bufs=2,
                                  name="ktc")
                    nc.sync.dma_start(ktc[:], ccKo[:, c, :, :].rearrange(
                        "r p t -> p r t"))
                    pc0 = ps.tile([DH + 1, TOK], f32, tag="ctx", bufs=2,
                                  name="pc0")
                    pc1 = ps.tile([DH + 1, TOK], f32, tag="ctx", bufs=2,
                                  name="pc1")
                    for kc in range(NK):
                        r_, j = divmod(kc, 4)
                        ps0 = ps.tile([P, TOK], f32, tag="mm", bufs=4,
                                      name="ps0")
                        nc.tensor.matmul(
                            ps0[:], ktc[0:DH, r_, 128 * j:128 * (j + 1)],
                            QT[0:DH, c, :], start=True, stop=True)
                        e0 = sb.tile([P, TOK], bf16, tag="E", bufs=3,
                                     name="e0")
                        nc.scalar.activation(e0[:], ps0[:], AF.Exp,
                                             scale=SCALE)
                        nc.tensor.matmul(pc0[:], v_sb[:, r_, j, 2 * c, :],
                                         e0[:], start=(kc == 0),
                                         stop=(kc == NK - 1))
                        ps1 = ps.tile([P, TOK], f32, tag="mm", bufs=4,
                                      name="ps1")
                        nc.tensor.matmul(
                            ps1[:], ktc[DH:P, r_, 128 * j:128 * (j + 1)],
                            QT[DH:P, c, :], start=True, stop=True)
                        e1 = sb.tile([P, TOK], bf16, tag="E", bufs=3,
                                     name="e1")
                        nc.scalar.activation(e1[:], ps1[:], AF.Exp,
                                             scale=SCALE)
                        nc.tensor.matmul(pc1[:], v_sb[:, r_, j, 2 * c + 1, :],
                                         e1[:], start=(kc == 0),
                                         stop=(kc == NK - 1))
                    for h, pch in ((0, pc0), (1, pc1)):
                        rec = sb.tile([1, TOK], f32r, tag="vec", bufs=4,
                                      name="rec")
                        with nc.allow_low_precision("softmax denom bcast"):
                            nc.vector.reciprocal(rec[:], pch[DH:DH + 1, :])
                        prr = ps.tile([DH, TOK], f32, tag="mm", bufs=4,
                                      name="prr")
                        nc.tensor.matmul(prr[:], ones[0:1, 0:DH], rec[:],
                                         start=True, stop=True)
                        rr = sb.tile([DH, TOK], f32, tag="rrep", bufs=2,
                                     name="rr")
                        nc.scalar.copy(rr[:], prr[:])
                        nc.vector.tensor_mul(
                            ctxT[DH * h:DH * (h + 1), c, :],
                            pch[0:DH, :], rr[:])

                # ---------------- output projection + residual -------------
                t1a = sb.tile([P, DC, TOK], f32r, tag="big2", bufs=2,
                              name="t1a")
                for g in range(4):
                    wt = wtile(wp_d[l, :, 256 * g:256 * (g + 1)], "wpt")
                    for cc in range(2):
                        c = 2 * g + cc
                        pp = ps.tile([P, TOK], f32, tag="mm", bufs=4,
                                     name="pp")
                        for kc in range(DC):
                            nc.tensor.matmul(
                                pp[:], wt[:, kc, 128 * cc:128 * (cc + 1)],
                                ctxT[:, kc, :],
                                start=(kc == 0), stop=(kc == DC - 1))
                        nc.vector.scalar_tensor_tensor(
                            out=t1a[:, c, :], in0=pp[:],
                            scalar=bp_sb[:, l, c:c + 1],
                            in1=xT[:, c, :].bitcast(dt.float32),
                            op0=OP.add, op1=OP.add)

                xmid = sb.tile([P, DC, TOK], f32r, tag="xT", bufs=2,
                               name="xmid")
                layernorm(l, t1a, g1_sb, be1_sb, xmid)

                # ---------------- FFN --------------------------------------
                t1f = sb.tile([P, DC, TOK], f32r, tag="big2", bufs=2,
                              name="t1f")
                for q in range(4):
                    hT = sb.tile([P, DC, TOK], f32r, tag="hT", bufs=1,
                                 name="hT")
                    for g in range(4):
                        col0 = q * 1024 + 256 * g
                        wt = wtile(w1_d[l, :, col0:col0 + 256], "w1t")
                        for cc in range(2):
                            fcl = 2 * g + cc
                            ph = ps.tile([P, TOK], f32, tag="mm", bufs=4,
                                         name="ph")
                            for kc in range(DC):
                                nc.tensor.matmul(
                                    ph[:], wt[:, kc, 128 * cc:128 * (cc + 1)],
                                    xmid[:, kc, :],
                                    start=(kc == 0), stop=(kc == DC - 1))
                            fcg = q * DC + fcl
                            nc.scalar.activation(
                                hT[:, fcl, :], ph[:], AF.Relu,
                                bias=b1_sb[:, l, fcg:fcg + 1])
                    for gc in range(4):
                        w2t = wtile(
                            w2_d[l, q * 1024:(q + 1) * 1024,
                                 256 * gc:256 * (gc + 1)], "w2t")
                        for cc in range(2):
                            c = 2 * gc + cc
                            py = ps.tile([P, TOK], f32, tag="mm", bufs=4,
                                         name="py")
                            for kc in range(DC):
                                nc.tensor.matmul(
                                    py[:],
                                    w2t[:, kc, 128 * cc:128 * (cc + 1)],
                                    hT[:, kc, :],
                                    start=(kc == 0), stop=(kc == DC - 1))
                            if q == 0:
                                nc.vector.scalar_tensor_tensor(
                                    out=t1f[:, c, :], in0=py[:],
                                    scalar=b2_sb[:, l, c:c + 1],
                                    in1=xmid[:, c, :].bitcast(dt.float32),
                                    op0=OP.add, op1=OP.add)
                            else:
                                nc.vector.tensor_add(
                                    t1f[:, c, :], py[:],
                                    t1f[:, c, :].bitcast(dt.float32))

                xnext = sb.tile([P, DC, TOK], f32r, tag="xT", bufs=2,
                                name="xnext")
                layernorm(l, t1f, g2_sb, be2_sb, xnext)
                xT = xnext

            nc.sync.dma_start(
                out_d[:, :].rearrange("(c p) t -> p c t", p=P).bitcast(f32r),
                xT[:])
    nc.finalize()
    return nc


_NC_CACHE = {}


def get_nc(n_layers=L):
    if n_layers not in _NC_CACHE:
        _NC_CACHE[n_layers] = build(n_layers)
    return _NC_CACHE[n_layers]


def run(inputs, n_layers=L, trace=False):
    """inputs: the full setup_inputs() dict. Returns (out, BassKernelResults)."""
    hs = np.asarray(inputs["hidden_states"], np.float32)
    f = lambda k: np.ascontiguousarray(np.asarray(inputs[k], np.float32))
    Wq, Wk, Wv, Wp = f("Wq"), f("Wk"), f("Wv"), f("Wp")
    W1, W2 = f("W1"), f("W2")
    bq, bk, bv, bp = f("bq"), f("bk"), f("bv"), f("bp")
    b1, b2 = f("b1"), f("b2")
    g1, be1, g2, be2 = f("ln1_g"), f("ln1_b"), f("ln2_g"), f("ln2_b")
    # fold the V bias through the output projection: P(V + 1 bv^T) Wp + bp
    # = P V Wp + r*(bv Wp) + bp, and after normalization r/r = 1.
    bp_eff = (bp + np.einsum("ld,ldo->lo", bv, Wp)).astype(np.float32)

    xflat = hs.reshape(B * S, D)
    in_maps = []
    for i in range(NCORES):
        xTi = np.ascontiguousarray(xflat[i * TOK:(i + 1) * TOK].T)
        in_maps.append(dict(
            xT=xTi, wq=Wq[:n_layers], wk=Wk[:n_layers], wv=Wv[:n_layers],
            wp=Wp[:n_layers], w1=W1[:n_layers], w2=W2[:n_layers],
            bq=bq[:n_layers], bk=bk[:n_layers], bp=bp_eff[:n_layers],
            b1=b1[:n_layers], b2=b2[:n_layers], g1=g1[:n_layers],
            be1=be1[:n_layers], g2=g2[:n_layers], be2=be2[:n_layers]))
    nc = get_nc(n_layers)
    res = bass_utils.run_bass_kernel_spmd(
        nc, in_maps, core_ids=list(range(NCORES)), trace=trace)
    out = np.empty((B * S, D), np.float32)
    for i in range(NCORES):
        out[i * TOK:(i + 1) * TOK] = res.results[i]["outT"].T
    return out.reshape(B, S, D), res


def kernel(**inputs):
    out, _ = run(inputs)
    return out


# revision 19
# speedup vs baseline: 1.0625x; 1.0625x over previous
"""4-layer transformer encoder (B=2, S=2048, D=1024, FF=4096, H=16) on 8 TRN2
NeuronCores.

Sharding: 4096 tokens split 512/core (cores 0-3 = batch 0, 4-7 = batch 1).
Weights replicated (host pre-tiles them so every weight DMA is contiguous).
Per layer: local QKV projections in fp32r, AllGather of K^T/V (bf16) within
each 4-core quad, attention computed as E^T = exp(K.Q^T/sqrt(dh)) with the
softmax denominator coming from a ones-augmented V matmul (the ones column is
staged into the gather payload), deferred normalization with one batched
reciprocal per layer, output projection + residual + LayerNorm
(partition-axis stats via ones-matmuls), then FFN + residual + LayerNorm.

Activations live transposed in SBUF (x^T: [D partitions, tokens free]) so no
on-device transposes are needed anywhere; the host transposes the input shard
once and the output shard back.
"""
import sys
if '/opt/trn_rl_repo' not in sys.path:
    sys.path.insert(0, '/opt/trn_rl_repo')

import numpy as np

import concourse.bass as bass
import concourse.mybir as mybir
import concourse.tile as tile
import concourse.bacc as bacc
from concourse import bass_utils

# problem config (hardcoded per contest rules)
L = 4
D = 1024
FF = 4096
H = 16
DH = 64
B = 2
S = 2048
EPS = 1e-6
SCALE = 1.0 / 8.0  # 1/sqrt(DH)

NCORES = 8
TOK = 512           # tokens per core
P = 128
DC = D // P         # 8 d-chunks
FC = FF // P        # 32 ff-chunks
NK = S // P         # 16 k-token chunks
R = 4               # ranks per quad (cores sharing one batch element)
RGROUPS = [[0, 1, 2, 3], [4, 5, 6, 7]]
HA = DH + 1         # V head block augmented with a ones column

dt = mybir.dt
AF = mybir.ActivationFunctionType
OP = mybir.AluOpType


def build(n_layers=L):
    nc = bacc.Bacc("TRN2", target_bir_lowering=False, debug=False,
                   num_devices=NCORES)
    f32, f32r, bf16 = dt.float32, dt.float32r, dt.bfloat16

    xT_d = nc.dram_tensor("xT", [D, TOK], f32, kind="ExternalInput")
    # weights pre-tiled on host: [n_layers, G, P, DC, 256]
    wq_d = nc.dram_tensor("wq", [n_layers, 4, P, DC, 256], f32,
                          kind="ExternalInput")
    wk_d = nc.dram_tensor("wk", [n_layers, 4, P, DC, 256], f32,
                          kind="ExternalInput")
    wv_d = nc.dram_tensor("wv", [n_layers, 4, P, DC, 256], f32,
                          kind="ExternalInput")
    wp_d = nc.dram_tensor("wp", [n_layers, 4, P, DC, 256], f32,
                          kind="ExternalInput")
    w1_d = nc.dram_tensor("w1", [n_layers, 16, P, DC, 256], f32,
                          kind="ExternalInput")
    w2_d = nc.dram_tensor("w2", [n_layers, 16, P, DC, 256], f32,
                          kind="ExternalInput")
    bq_d = nc.dram_tensor("bq", [n_layers, D], f32, kind="ExternalInput")
    bk_d = nc.dram_tensor("bk", [n_layers, D], f32, kind="ExternalInput")
    bp_d = nc.dram_tensor("bp", [n_layers, D], f32, kind="ExternalInput")
    b1_d = nc.dram_tensor("b1", [n_layers, FF], f32, kind="ExternalInput")
    b2_d = nc.dram_tensor("b2", [n_layers, D], f32, kind="ExternalInput")
    g1_d = nc.dram_tensor("g1", [n_layers, D], f32, kind="ExternalInput")
    be1_d = nc.dram_tensor("be1", [n_layers, D], f32, kind="ExternalInput")
    g2_d = nc.dram_tensor("g2", [n_layers, D], f32, kind="ExternalInput")
    be2_d = nc.dram_tensor("be2", [n_layers, D], f32, kind="ExternalInput")
    out_d = nc.dram_tensor("outT", [D, TOK], f32, kind="ExternalOutput")

    with tile.TileContext(nc) as tc:
        with (
            tc.tile_pool(name="pers", bufs=1) as pers,
            tc.tile_pool(name="sb", bufs=1) as sb,
            tc.tile_pool(name="ps", bufs=1, space="PSUM") as ps,
            tc.tile_pool(name="dram", bufs=1, space="DRAM") as dram,
        ):
            ones_f = pers.tile([P, P], f32)
            nc.vector.memset(ones_f[:], 1.0)
            ones = pers.tile([P, P], f32r)
            nc.vector.tensor_copy(out=ones[:], in_=ones_f[:])
            eps_sb = pers.tile([1, 1], f32)
            nc.vector.memset(eps_sb[:], EPS)


            def load_param(name, src, nchunk):
                t = pers.tile([P, n_layers, nchunk], f32, name=name)
                nc.sync.dma_start(
                    t[:], src[:, :].rearrange("l (c p) -> p l c", p=P))
                return t

            bq_sb = load_param("bq_sb", bq_d, DC)
            bk_sb = load_param("bk_sb", bk_d, DC)
            bp_sb = load_param("bp_sb", bp_d, DC)
            b2_sb = load_param("b2_sb", b2_d, DC)
            g1_sb = load_param("g1_sb", g1_d, DC)
            be1_sb = load_param("be1_sb", be1_d, DC)
            g2_sb = load_param("g2_sb", g2_d, DC)
            be2_sb = load_param("be2_sb", be2_d, DC)
            b1_sb = load_param("b1_sb", b1_d, FC)

            xT = sb.tile([P, DC, TOK], f32r, tag="xT", bufs=2, name="xT0")
            nc.sync.dma_start(
                xT[:],
                xT_d[:, :].rearrange("(c p) t -> p c t", p=P).bitcast(f32r))

            def wtile(w_d, l, g, name):
                t = sb.tile([P, DC, 256], f32r, tag="wblk", bufs=2, name=name)
                nc.sync.dma_start(t[:], w_d[l, g].bitcast(f32r))
                return t

            def layernorm(l, t1, g_sb, be_sb, xout):
                """xout[:, c, :] = LN(t1) over the partition (d) axis."""
                psum_s = ps.tile([1, TOK], f32, tag="stat", bufs=2,
                                 name="psum_s")
                psum_sq = ps.tile([1, TOK], f32, tag="stat", bufs=2,
                                  name="psum_sq")
                for c in range(DC):
                    nc.tensor.matmul(psum_s[:], ones[:, 0:1], t1[:, c, :],
                                     start=(c == 0), stop=(c == DC - 1))
                for c in range(DC):
                    sqc = sb.tile([P, TOK], f32r, tag="sq", bufs=2, name="sqc")
                    nc.scalar.square(sqc[:], t1[:, c, :])
                    nc.tensor.matmul(psum_sq[:], ones[:, 0:1], sqc[:],
                                     start=(c == 0), stop=(c == DC - 1))
                mean = sb.tile([1, TOK], f32r, tag="vec", bufs=4, name="mean")
                nc.vector.tensor_scalar_mul(mean[:], psum_s[:], 1.0 / D)
                ms = sb.tile([1, TOK], f32, tag="vec", bufs=4, name="ms")
                nc.vector.tensor_scalar_mul(ms[:], psum_sq[:], 1.0 / D)
                var = sb.tile([1, TOK], f32, tag="vec", bufs=4, name="var")
                # var = ms - mean*mean = (mean * -mean) * mean + ms
                nc.vector.scalar_tensor_tensor(
                    out=var[:], in0=mean[:].bitcast(f32), scalar=-1.0,
                    in1=mean[:].bitcast(f32), op0=OP.mult, op1=OP.mult)
                nc.vector.tensor_sub(var[:], ms[:], var[:])
                std = sb.tile([1, TOK], f32, tag="vec", bufs=4, name="std")
                nc.scalar.activation(std[:], var[:], AF.Sqrt, bias=eps_sb[:])
                rstd = sb.tile([1, TOK], f32r, tag="vec", bufs=4, name="rstd")
                with nc.allow_low_precision("fp32r rstd for PE broadcast"):
                    nc.vector.reciprocal(rstd[:], std[:])
                pm = ps.tile([P, TOK], f32, tag="mm", bufs=4, name="pm")
                nc.tensor.matmul(pm[:], ones[0:1, :], mean[:],
                                 start=True, stop=True)
                mrep = sb.tile([P, TOK], f32, tag="mrep", bufs=1, name="mrep")
                nc.scalar.copy(mrep[:], pm[:])
                pr = ps.tile([P, TOK], f32, tag="mm", bufs=4, name="pr")
                nc.tensor.matmul(pr[:], ones[0:1, :], rstd[:],
                                 start=True, stop=True)
                rrep = sb.tile([P, TOK], f32, tag="rrepLN", bufs=1,
                               name="rrep")
                nc.scalar.copy(rrep[:], pr[:])
                for c in range(DC):
                    d1 = sb.tile([P, TOK], f32, tag="lnscr", bufs=3,
                                 name="d1")
                    nc.vector.tensor_sub(d1[:], t1[:, c, :].bitcast(f32),
                                         mrep[:])
                    d2 = sb.tile([P, TOK], f32, tag="lnscr", bufs=3,
                                 name="d2")
                    nc.vector.tensor_mul(d2[:], d1[:], rrep[:])
                    nc.vector.tensor_scalar(
                        out=xout[:, c, :], in0=d2[:],
                        scalar1=g_sb[:, l, c:c + 1],
                        scalar2=be_sb[:, l, c:c + 1],
                        op0=OP.mult, op1=OP.add)

            for l in range(n_layers):
                # ---------------- K projection (staged to gather input) ----
                ccK = dram.tile([DC, P, TOK], bf16, tag="ccK", bufs=2,
                                name="ccK")
                for g in range(4):
                    wt = wtile(wk_d, l, g, "wkt")
                    for cc in range(2):
                        c = 2 * g + cc
                        pk = ps.tile([P, TOK], f32, tag="mm", bufs=4,
                                     name="pk")
                        for kc in range(DC):
                            nc.tensor.matmul(
                                pk[:], wt[:, kc, 128 * cc:128 * (cc + 1)],
                                xT[:, kc, :],
                                start=(kc == 0), stop=(kc == DC - 1))
                        kst = sb.tile([P, TOK], bf16, tag="kvstage", bufs=2,
                                      name="kst")
                        nc.scalar.activation(kst[:], pk[:], AF.Identity,
                                             bias=bk_sb[:, l, c:c + 1])
                        nc.sync.dma_start(ccK[c, :, :], kst[:])
                ccKo = dram.tile([R, DC, P, TOK], bf16, tag="ccKo", bufs=2,
                                 name="ccKo")
                nc.gpsimd.collective_compute(
                    "AllGather", OP.bypass, replica_groups=RGROUPS,
                    ins=[ccK[:].opt()], outs=[ccKo[:].opt()])

                # ---------------- V projection (token-major, augmented) ----
                # ccV[tc, p, h*HA + d] = V[tc*128+p, 64h+d]; col d=64 is 1.0
                ccV = dram.tile([4, P, H * HA], bf16, tag="ccV", bufs=2,
                                name="ccV")
                for g in range(4):
                    wt = wtile(wv_d, l, g, "wvt")
                    for tc in range(4):
                        pv = ps.tile([P, 256], f32, tag="mm", bufs=4,
                                     name="pv")
                        for kc in range(DC):
                            nc.tensor.matmul(
                                pv[:], xT[:, kc, 128 * tc:128 * (tc + 1)],
                                wt[:, kc, :],
                                start=(kc == 0), stop=(kc == DC - 1))
                        # 256 cols = heads 4g..4g+3; stage with ones column
                        vst = sb.tile([P, 4, HA], bf16, tag="vstage", bufs=2,
                                      name="vst")
                        nc.vector.tensor_copy(
                            out=vst[:, :, 0:DH],
                            in_=pv[:].rearrange("p (h d) -> p h d", d=DH))
                        nc.vector.memset(vst[:, :, DH:HA], 1.0)
                        nc.sync.dma_start(
                            ccV[tc, :, 4 * g * HA:(4 * g + 4) * HA],
                            vst[:].rearrange("p h d -> p (h d)"))
                ccVo = dram.tile([R, 4, P, H * HA], bf16, tag="ccVo", bufs=2,
                                 name="ccVo")
                nc.gpsimd.collective_compute(
                    "AllGather", OP.bypass, replica_groups=RGROUPS,
                    ins=[ccV[:].opt()], outs=[ccVo[:].opt()])

                # ---------------- Q projection (stays local, bf16) ---------
                QT = sb.tile([P, DC, TOK], bf16, tag="QT", bufs=1, name="QT")
                for g in range(4):
                    wt = wtile(wq_d, l, g, "wqt")
                    for cc in range(2):
                        c = 2 * g + cc
                        pq = ps.tile([P, TOK], f32, tag="mm", bufs=4,
                                     name="pq")
                        for kc in range(DC):
                            nc.tensor.matmul(
                                pq[:], wt[:, kc, 128 * cc:128 * (cc + 1)],
                                xT[:, kc, :],
                                start=(kc == 0), stop=(kc == DC - 1))
                        nc.scalar.activation(QT[:, c, :], pq[:], AF.Identity,
                                             bias=bq_sb[:, l, c:c + 1])

                # ---------------- gathered K/V into SBUF -------------------
                v_sb = sb.tile([P, R, 4, H * HA], bf16, tag="Vg", bufs=1,
                               name="v_sb")
                for r_ in range(R):
                    for tc in range(4):
                        nc.sync.dma_start(v_sb[:, r_, tc, :], ccVo[r_, tc])

                # ---------------- attention, head pair per c ---------------
                # ctx~ (unnormalized) and per-head denominators r; one
                # batched reciprocal at the end of the phase.
                ctxT = sb.tile([P, DC, TOK], f32r, tag="big2", bufs=2,
                               name="ctxT")
                for c in range(DC):
                    ktc = sb.tile([P, R, TOK], bf16, tag="KTc", bufs=2,
                                  name="ktc")
                    nc.sync.dma_start(ktc[:], ccKo[:, c, :, :].rearrange(
                        "r p t -> p r t"))
                    pc0 = ps.tile([HA, TOK], f32, tag="ctx", bufs=2,
                                  name="pc0")
                    pc1 = ps.tile([HA, TOK], f32, tag="ctx", bufs=2,
                                  name="pc1")
                    for kc in range(NK):
                        r_, j = divmod(kc, 4)
                        ps0 = ps.tile([P, TOK], f32, tag="mm", bufs=4,
                                      name="ps0")
                        nc.tensor.matmul(
                            ps0[:], ktc[0:DH, r_, 128 * j:128 * (j + 1)],
                            QT[0:DH, c, :], start=True, stop=True)
                        eb0 = sb.tile([P, TOK], bf16, tag="E", bufs=3,
                                      name="eb0")
                        nc.vector.tensor_copy(out=eb0[:], in_=ps0[:])
                        e0 = sb.tile([P, TOK], bf16, tag="E2", bufs=3,
                                     name="e0")
                        nc.scalar.activation(e0[:], eb0[:], AF.Exp,
                                             scale=SCALE)
                        nc.tensor.matmul(
                            pc0[:], v_sb[:, r_, j, HA * 2 * c:HA * 2 * c + HA],
                            e0[:], start=(kc == 0), stop=(kc == NK - 1))
                        ps1 = ps.tile([P, TOK], f32, tag="mm", bufs=4,
                                      name="ps1")
                        nc.tensor.matmul(
                            ps1[:], ktc[DH:P, r_, 128 * j:128 * (j + 1)],
                            QT[DH:P, c, :], start=True, stop=True)
                        eb1 = sb.tile([P, TOK], bf16, tag="E", bufs=3,
                                      name="eb1")
                        nc.vector.tensor_copy(out=eb1[:], in_=ps1[:])
                        e1 = sb.tile([P, TOK], bf16, tag="E2", bufs=3,
                                     name="e1")
                        nc.scalar.activation(e1[:], eb1[:], AF.Exp,
                                             scale=SCALE)
                        nc.tensor.matmul(
                            pc1[:],
                            v_sb[:, r_, j, HA * (2 * c + 1):
                                 HA * (2 * c + 1) + HA],
                            e1[:], start=(kc == 0), stop=(kc == NK - 1))
                    for h, pch in ((0, pc0), (1, pc1)):
                        # softmax denominator r sits on psum partition 64;
                        # spread it across 128 partitions via a DRAM bounce
                        # so the reciprocal runs 128-wide, then bring it back
                        # as a [1, TOK] row for the ones-broadcast matmul.
                        rst = sb.tile([HA, TOK], f32, tag="rst", bufs=2,
                                      name="rst")
                        nc.vector.tensor_copy(out=rst[DH:HA, :],
                                              in_=pch[DH:HA, :])
                        drT = dram.tile([TOK], f32, tag="drT", bufs=4,
                                        name="drT")
                        nc.sync.dma_start(drT[:], rst[DH:HA, :])
                        rT = sb.tile([P, TOK // P], f32, tag="rT", bufs=2,
                                     name="rT")
                        nc.sync.dma_start(
                            rT[:], drT[:].rearrange("(p f) -> p f", p=P))
                        rTr = sb.tile([P, TOK // P], f32r, tag="rT2", bufs=2,
                                      name="rTr")
                        with nc.allow_low_precision("softmax denominators"):
                            nc.vector.reciprocal(rTr[:], rT[:])
                        drT2 = dram.tile([TOK], f32r, tag="drT2", bufs=4,
                                         name="drT2")
                        nc.sync.dma_start(
                            drT2[:].rearrange("(p f) -> p f", p=P), rTr[:])
                        rrow = sb.tile([1, TOK], f32r, tag="rrow", bufs=2,
                                       name="rrow")
                        nc.sync.dma_start(rrow[:], drT2[:].rearrange(
                            "(o t) -> o t", o=1))
                        prr = ps.tile([DH, TOK], f32, tag="mm", bufs=4,
                                      name="prr")
                        nc.tensor.matmul(prr[:], ones[0:1, 0:DH], rrow[:],
                                         start=True, stop=True)
                        rr = sb.tile([DH, TOK], f32, tag="rrep", bufs=2,
                                     name="rr")
                        nc.scalar.copy(rr[:], prr[:])
                        nc.vector.tensor_mul(
                            ctxT[DH * h:DH * (h + 1), c, :],
                            pch[0:DH, :], rr[:])

                # ---------------- output projection + residual -------------
                t1a = sb.tile([P, DC, TOK], f32r, tag="big2", bufs=2,
                              name="t1a")
                for g in range(4):
                    wt = wtile(wp_d, l, g, "wpt")
                    for cc in range(2):
                        c = 2 * g + cc
                        pp = ps.tile([P, TOK], f32, tag="mm", bufs=4,
                                     name="pp")
                        for kc in range(DC):
                            nc.tensor.matmul(
                                pp[:], wt[:, kc, 128 * cc:128 * (cc + 1)],
                                ctxT[:, kc, :],
                                start=(kc == 0), stop=(kc == DC - 1))
                        nc.vector.scalar_tensor_tensor(
                            out=t1a[:, c, :], in0=pp[:],
                            scalar=bp_sb[:, l, c:c + 1],
                            in1=xT[:, c, :].bitcast(f32),
                            op0=OP.add, op1=OP.add)

                xmid = sb.tile([P, DC, TOK], f32r, tag="xT", bufs=2,
                               name="xmid")
                layernorm(l, t1a, g1_sb, be1_sb, xmid)

                # ---------------- FFN --------------------------------------
                t1f = sb.tile([P, DC, TOK], f32r, tag="big2", bufs=2,
                              name="t1f")
                for q in range(4):
                    hT = sb.tile([P, DC, TOK], f32r, tag="hT", bufs=1,
                                 name="hT")
                    for g in range(4):
                        wt = wtile(w1_d, l, 4 * q + g, "w1t")
                        for cc in range(2):
                            fcl = 2 * g + cc
                            ph = ps.tile([P, TOK], f32, tag="mm", bufs=4,
                                         name="ph")
                            for kc in range(DC):
                                nc.tensor.matmul(
                                    ph[:], wt[:, kc, 128 * cc:128 * (cc + 1)],
                                    xmid[:, kc, :],
                                    start=(kc == 0), stop=(kc == DC - 1))
                            fcg = q * DC + fcl
                            nc.scalar.activation(
                                hT[:, fcl, :], ph[:], AF.Relu,
                                bias=b1_sb[:, l, fcg:fcg + 1])
                    for gc in range(4):
                        w2t = wtile(w2_d, l, 4 * q + gc, "w2t")
                        for cc in range(2):
                            c = 2 * gc + cc
                            py = ps.tile([P, TOK], f32, tag="mm", bufs=4,
                                         name="py")
                            for kc in range(DC):
                                nc.tensor.matmul(
                                    py[:],
                                    w2t[:, kc, 128 * cc:128 * (cc + 1)],
                                    hT[:, kc, :],
                                    start=(kc == 0), stop=(kc == DC - 1))
                            if q == 0:
                                nc.vector.scalar_tensor_tensor(
                                    out=t1f[:, c, :], in0=py[:],
                                    scalar=b2_sb[:, l, c:c + 1],
                                    in1=xmid[:, c, :].bitcast(f32),
                                    op0=OP.add, op1=OP.add)
                            else:
                                nc.vector.tensor_add(
                                    t1f[:, c, :], py[:],
                                    t1f[:, c, :].bitcast(f32))

                xnext = sb.tile([P, DC, TOK], f32r, tag="xT", bufs=2,
                                name="xnext")
                layernorm(l, t1f, g2_sb, be2_sb, xnext)
                xT = xnext

            nc.sync.dma_start(
                out_d[:, :].rearrange("(c p) t -> p c t", p=P).bitcast(f32r),
                xT[:])
    nc.finalize()
    return nc


_NC_CACHE = {}


def get_nc(n_layers=L):
    if n_layers not in _NC_CACHE:
        _NC_CACHE[n_layers] = build(n_layers)
    return _NC_CACHE[n_layers]


def _tile_weight(w, G):
    """[L, K, O] -> [L, G_total, P, K//128, 256] matching wtile() blocks.

    For K=D (projections): block g covers out-cols 256g..256g+256, all K.
    For W1/W2 the same formula applies per 1024-col quarter group because
    blocks are indexed 4q+g and cover kc-chunks of the full K dim for W1,
    and kc-local chunks for W2 (handled by the caller's slicing)."""
    Lw, K, O = w.shape
    t = w.reshape(Lw, K // P, P, O // 256, 256).transpose(0, 3, 2, 1, 4)
    return np.ascontiguousarray(t)


def _tile_w2(w2):
    """[L, FF, D] -> [L, 16, P, 8, 256]; block 4q+gc covers W2 rows
    1024q..1024(q+1), cols 256gc..256(gc+1)."""
    Lw = w2.shape[0]
    t = w2.reshape(Lw, 4, 8, P, 4, 256).transpose(0, 1, 4, 3, 2, 5)
    return np.ascontiguousarray(t.reshape(Lw, 16, P, 8, 256))


def run(inputs, n_layers=L, trace=False):
    """inputs: the full setup_inputs() dict. Returns (out, BassKernelResults)."""
    hs = np.asarray(inputs["hidden_states"], np.float32)
    f = lambda k: np.ascontiguousarray(np.asarray(inputs[k], np.float32))
    Wq, Wk, Wv, Wp = f("Wq"), f("Wk"), f("Wv"), f("Wp")
    W1, W2 = f("W1"), f("W2")
    bq, bk, bv, bp = f("bq"), f("bk"), f("bv"), f("bp")
    b1, b2 = f("b1"), f("b2")
    g1, be1, g2, be2 = f("ln1_g"), f("ln1_b"), f("ln2_g"), f("ln2_b")
    # fold the V bias through the output projection: P(V + 1 bv^T) Wp + bp
    # = P V Wp + r*(bv Wp) + bp, and after normalization r/r = 1.
    bp_eff = (bp + np.einsum("ld,ldo->lo", bv, Wp)).astype(np.float32)

    wq_t = _tile_weight(Wq[:n_layers], 4)
    wk_t = _tile_weight(Wk[:n_layers], 4)
    wv_t = _tile_weight(Wv[:n_layers], 4)
    wp_t = _tile_weight(Wp[:n_layers], 4)
    w1_t = _tile_weight(W1[:n_layers], 16)
    w2_t = _tile_w2(W2[:n_layers])

    xflat = hs.reshape(B * S, D)
    in_maps = []
    for i in range(NCORES):
        xTi = np.ascontiguousarray(xflat[i * TOK:(i + 1) * TOK].T)
        in_maps.append(dict(
            xT=xTi,
            wq=wq_t, wk=wk_t, wv=wv_t, wp=wp_t, w1=w1_t, w2=w2_t,
            bq=bq[:n_layers], bk=bk[:n_layers], bp=bp_eff[:n_layers],
            b1=b1[:n_layers], b2=b2[:n_layers], g1=g1[:n_layers],
            be1=be1[:n_layers], g2=g2[:n_layers], be2=be2[:n_layers]))
    nc = get_nc(n_layers)
    res = bass_utils.run_bass_kernel_spmd(
        nc, in_maps, core_ids=list(range(NCORES)), trace=trace)
    out = np.empty((B * S, D), np.float32)
    for i in range(NCORES):
        out[i * TOK:(i + 1) * TOK] = res.results[i]["outT"].T
    return out.reshape(B, S, D), res


def kernel(**inputs):
    out, _ = run(inputs)
    return out


# revision 21
# speedup vs baseline: 1.2189x; 1.1473x over previous
"""4-layer transformer encoder (B=2, S=2048, D=1024, FF=4096, H=16) on 8 TRN2
NeuronCores.

Sharding: 4096 tokens split 512/core (cores 0-3 = batch 0, 4-7 = batch 1).
Weights replicated (host pre-tiles them so every weight DMA is contiguous).
Per layer: local QKV projections in fp32r, AllGather of K^T/V (bf16) within
each 4-core quad, attention computed as E^T = exp(K.Q^T/sqrt(dh)) with the
softmax denominator coming from a ones-augmented V matmul (the ones column is
staged into the gather payload), deferred normalization with one batched
reciprocal per layer, output projection + residual + LayerNorm
(partition-axis stats via ones-matmuls), then FFN + residual + LayerNorm.

Activations live transposed in SBUF (x^T: [D partitions, tokens free]) so no
on-device transposes are needed anywhere; the host transposes the input shard
once and the output shard back.
"""
import sys
if '/opt/trn_rl_repo' not in sys.path:
    sys.path.insert(0, '/opt/trn_rl_repo')

import numpy as np

import concourse.bass as bass
import concourse.mybir as mybir
import concourse.tile as tile
import concourse.bacc as bacc
from concourse import bass_utils

# problem config (hardcoded per contest rules)
L = 4
D = 1024
FF = 4096
H = 16
DH = 64
B = 2
S = 2048
EPS = 1e-6
SCALE = 1.0 / 8.0  # 1/sqrt(DH)

NCORES = 8
TOK = 512           # tokens per core
P = 128
DC = D // P         # 8 d-chunks
FC = FF // P        # 32 ff-chunks
NK = S // P         # 16 k-token chunks
R = 4               # ranks per quad (cores sharing one batch element)
RGROUPS = [[0, 1, 2, 3], [4, 5, 6, 7]]
HA = DH + 1         # V head block augmented with a ones column

dt = mybir.dt
AF = mybir.ActivationFunctionType
OP = mybir.AluOpType


def build(n_layers=L):
    nc = bacc.Bacc("TRN2", target_bir_lowering=False, debug=False,
                   num_devices=NCORES)
    f32, f32r, bf16 = dt.float32, dt.float32r, dt.bfloat16

    xT_d = nc.dram_tensor("xT", [D, TOK], f32, kind="ExternalInput")
    # weights pre-tiled on host: [n_layers, G, P, DC, 256]
    wq_d = nc.dram_tensor("wq", [n_layers, 4, P, DC, 256], f32,
                          kind="ExternalInput")
    wk_d = nc.dram_tensor("wk", [n_layers, 4, P, DC, 256], f32,
                          kind="ExternalInput")
    wv_d = nc.dram_tensor("wv", [n_layers, 4, P, DC, 256], f32,
                          kind="ExternalInput")
    wp_d = nc.dram_tensor("wp", [n_layers, 4, P, DC, 256], f32,
                          kind="ExternalInput")
    w1_d = nc.dram_tensor("w1", [n_layers, 16, P, DC, 256], f32,
                          kind="ExternalInput")
    w2_d = nc.dram_tensor("w2", [n_layers, 16, P, DC, 256], f32,
                          kind="ExternalInput")
    bq_d = nc.dram_tensor("bq", [n_layers, D], f32, kind="ExternalInput")
    bk_d = nc.dram_tensor("bk", [n_layers, D], f32, kind="ExternalInput")
    bp_d = nc.dram_tensor("bp", [n_layers, D], f32, kind="ExternalInput")
    b1_d = nc.dram_tensor("b1", [n_layers, FF], f32, kind="ExternalInput")
    b2_d = nc.dram_tensor("b2", [n_layers, D], f32, kind="ExternalInput")
    g1_d = nc.dram_tensor("g1", [n_layers, D], f32, kind="ExternalInput")
    be1_d = nc.dram_tensor("be1", [n_layers, D], f32, kind="ExternalInput")
    g2_d = nc.dram_tensor("g2", [n_layers, D], f32, kind="ExternalInput")
    be2_d = nc.dram_tensor("be2", [n_layers, D], f32, kind="ExternalInput")
    out_d = nc.dram_tensor("outT", [D, TOK], f32, kind="ExternalOutput")

    with tile.TileContext(nc) as tc:
        with (
            tc.tile_pool(name="pers", bufs=1) as pers,
            tc.tile_pool(name="sb", bufs=1) as sb,
            tc.tile_pool(name="ps", bufs=1, space="PSUM") as ps,
            tc.tile_pool(name="dram", bufs=1, space="DRAM") as dram,
        ):
            ones_f = pers.tile([P, P], f32)
            nc.vector.memset(ones_f[:], 1.0)
            ones = pers.tile([P, P], f32r)
            nc.vector.tensor_copy(out=ones[:], in_=ones_f[:])
            eps_sb = pers.tile([1, 1], f32)
            nc.vector.memset(eps_sb[:], EPS)


            def load_param(name, src, nchunk):
                t = pers.tile([P, n_layers, nchunk], f32, name=name)
                nc.sync.dma_start(
                    t[:], src[:, :].rearrange("l (c p) -> p l c", p=P))
                return t

            bq_sb = load_param("bq_sb", bq_d, DC)
            bk_sb = load_param("bk_sb", bk_d, DC)
            bp_sb = load_param("bp_sb", bp_d, DC)
            b2_sb = load_param("b2_sb", b2_d, DC)
            g1_sb = load_param("g1_sb", g1_d, DC)
            be1_sb = load_param("be1_sb", be1_d, DC)
            g2_sb = load_param("g2_sb", g2_d, DC)
            be2_sb = load_param("be2_sb", be2_d, DC)
            b1_sb = load_param("b1_sb", b1_d, FC)

            xT = sb.tile([P, DC, TOK], f32r, tag="xT", bufs=2, name="xT0")
            nc.sync.dma_start(
                xT[:],
                xT_d[:, :].rearrange("(c p) t -> p c t", p=P).bitcast(f32r))

            def wtile(w_d, l, g, name):
                t = sb.tile([P, DC, 256], f32r, tag="wblk", bufs=2, name=name)
                nc.sync.dma_start(t[:], w_d[l, g].bitcast(f32r))
                return t

            def layernorm(l, t1, g_sb, be_sb, xout):
                """xout[:, c, :] = LN(t1) over the partition (d) axis."""
                psum_s = ps.tile([1, TOK], f32, tag="mm", bufs=2,
                                 name="psum_s")
                psum_sq = ps.tile([1, TOK], f32, tag="mm", bufs=2,
                                  name="psum_sq")
                for c in range(DC):
                    nc.tensor.matmul(psum_s[:], ones[:, 0:1], t1[:, c, :],
                                     start=(c == 0), stop=(c == DC - 1))
                for c in range(DC):
                    sqc = sb.tile([P, TOK], f32r, tag="sq", bufs=2, name="sqc")
                    nc.scalar.square(sqc[:], t1[:, c, :])
                    nc.tensor.matmul(psum_sq[:], ones[:, 0:1], sqc[:],
                                     start=(c == 0), stop=(c == DC - 1))
                mean = sb.tile([1, TOK], f32r, tag="vec", bufs=4, name="mean")
                nc.vector.tensor_scalar_mul(mean[:], psum_s[:], 1.0 / D)
                ms = sb.tile([1, TOK], f32, tag="vec", bufs=4, name="ms")
                nc.vector.tensor_scalar_mul(ms[:], psum_sq[:], 1.0 / D)
                var = sb.tile([1, TOK], f32, tag="vec", bufs=4, name="var")
                # var = ms - mean*mean = (mean * -mean) * mean + ms
                nc.vector.scalar_tensor_tensor(
                    out=var[:], in0=mean[:].bitcast(f32), scalar=-1.0,
                    in1=mean[:].bitcast(f32), op0=OP.mult, op1=OP.mult)
                nc.vector.tensor_sub(var[:], ms[:], var[:])
                std = sb.tile([1, TOK], f32, tag="vec", bufs=4, name="std")
                nc.scalar.activation(std[:], var[:], AF.Sqrt, bias=eps_sb[:])
                rstd = sb.tile([1, TOK], f32r, tag="vec", bufs=4, name="rstd")
                with nc.allow_low_precision("fp32r rstd for PE broadcast"):
                    nc.vector.reciprocal(rstd[:], std[:])
                pm = ps.tile([P, TOK], f32, tag="mm", bufs=2, name="pm")
                nc.tensor.matmul(pm[:], ones[0:1, :], mean[:],
                                 start=True, stop=True)
                mrep = sb.tile([P, TOK], f32, tag="mrep", bufs=1, name="mrep")
                nc.scalar.copy(mrep[:], pm[:])
                pr = ps.tile([P, TOK], f32, tag="mm", bufs=2, name="pr")
                nc.tensor.matmul(pr[:], ones[0:1, :], rstd[:],
                                 start=True, stop=True)
                rrep = sb.tile([P, TOK], f32, tag="rrepLN", bufs=1,
                               name="rrep")
                nc.scalar.copy(rrep[:], pr[:])
                for c in range(DC):
                    d1 = sb.tile([P, TOK], f32, tag="lnscr", bufs=3,
                                 name="d1")
                    nc.vector.tensor_sub(d1[:], t1[:, c, :].bitcast(f32),
                                         mrep[:])
                    d2 = sb.tile([P, TOK], f32, tag="lnscr", bufs=3,
                                 name="d2")
                    nc.vector.tensor_mul(d2[:], d1[:], rrep[:])
                    nc.vector.tensor_scalar(
                        out=xout[:, c, :], in0=d2[:],
                        scalar1=g_sb[:, l, c:c + 1],
                        scalar2=be_sb[:, l, c:c + 1],
                        op0=OP.mult, op1=OP.add)

            for l in range(n_layers):
                # ---------------- K projection (staged to gather input) ----
                ccK = dram.tile([DC, P, TOK], bf16, tag="ccK", bufs=2,
                                name="ccK")
                for g in range(4):
                    wt = wtile(wk_d, l, g, "wkt")
                    for cc in range(2):
                        c = 2 * g + cc
                        pk = ps.tile([P, TOK], f32, tag="mm", bufs=2,
                                     name="pk")
                        for kc in range(DC):
                            nc.tensor.matmul(
                                pk[:], wt[:, kc, 128 * cc:128 * (cc + 1)],
                                xT[:, kc, :],
                                start=(kc == 0), stop=(kc == DC - 1))
                        kst = sb.tile([P, TOK], bf16, tag="kvstage", bufs=2,
                                      name="kst")
                        nc.scalar.activation(kst[:], pk[:], AF.Identity,
                                             bias=bk_sb[:, l, c:c + 1])
                        nc.sync.dma_start(ccK[c, :, :], kst[:])
                ccKo = dram.tile([R, DC, P, TOK], bf16, tag="ccKo", bufs=2,
                                 name="ccKo")
                nc.gpsimd.collective_compute(
                    "AllGather", OP.bypass, replica_groups=RGROUPS,
                    ins=[ccK[:].opt()], outs=[ccKo[:].opt()])

                # ---------------- V projection (token-major, augmented) ----
                # ccV[tc, p, h*HA + d] = V[tc*128+p, 64h+d]; col d=64 is 1.0
                ccV = dram.tile([4, P, H * HA], bf16, tag="ccV", bufs=2,
                                name="ccV")
                for g in range(4):
                    wt = wtile(wv_d, l, g, "wvt")
                    for tc in range(4):
                        pv = ps.tile([P, 256], f32, tag="mm", bufs=2,
                                     name="pv")
                        for kc in range(DC):
                            nc.tensor.matmul(
                                pv[:], xT[:, kc, 128 * tc:128 * (tc + 1)],
                                wt[:, kc, :],
                                start=(kc == 0), stop=(kc == DC - 1))
                        # 256 cols = heads 4g..4g+3; stage with ones column
                        vst = sb.tile([P, 4, HA], bf16, tag="vstage", bufs=2,
                                      name="vst")
                        nc.vector.tensor_copy(
                            out=vst[:, :, 0:DH],
                            in_=pv[:].rearrange("p (h d) -> p h d", d=DH))
                        nc.vector.memset(vst[:, :, DH:HA], 1.0)
                        nc.sync.dma_start(
                            ccV[tc, :, 4 * g * HA:(4 * g + 4) * HA],
                            vst[:].rearrange("p h d -> p (h d)"))
                ccVo = dram.tile([R, 4, P, H * HA], bf16, tag="ccVo", bufs=2,
                                 name="ccVo")
                nc.gpsimd.collective_compute(
                    "AllGather", OP.bypass, replica_groups=RGROUPS,
                    ins=[ccV[:].opt()], outs=[ccVo[:].opt()])

                # ---------------- Q projection (stays local, bf16) ---------
                QT = sb.tile([P, DC, TOK], bf16, tag="QT", bufs=1, name="QT")
                for g in range(4):
                    wt = wtile(wq_d, l, g, "wqt")
                    for cc in range(2):
                        c = 2 * g + cc
                        pq = ps.tile([P, TOK], f32, tag="mm", bufs=2,
                                     name="pq")
                        for kc in range(DC):
                            nc.tensor.matmul(
                                pq[:], wt[:, kc, 128 * cc:128 * (cc + 1)],
                                xT[:, kc, :],
                                start=(kc == 0), stop=(kc == DC - 1))
                        nc.scalar.activation(QT[:, c, :], pq[:], AF.Identity,
                                             bias=bq_sb[:, l, c:c + 1])

                # ---------------- gathered K/V into SBUF -------------------
                v_sb = sb.tile([P, R, 4, H * HA], bf16, tag="Vg", bufs=1,
                               name="v_sb")
                for r_ in range(R):
                    for tc in range(4):
                        nc.sync.dma_start(v_sb[:, r_, tc, :], ccVo[r_, tc])

                # ---------------- attention, head pair per c ---------------
                # ctx~ (unnormalized) and per-head denominators r; one
                # batched reciprocal at the end of the phase.
                ctxT = sb.tile([P, DC, TOK], f32r, tag="big2", bufs=2,
                               name="ctxT")
                for c in range(DC):
                    ktc = sb.tile([P, R, TOK], bf16, tag="KTc", bufs=2,
                                  name="ktc")
                    nc.sync.dma_start(ktc[:], ccKo[:, c, :, :].rearrange(
                        "r p t -> p r t"))
                    pc0 = ps.tile([HA, TOK], f32, tag="ctx", bufs=2,
                                  name="pc0")
                    pc1 = ps.tile([HA, TOK], f32, tag="ctx", bufs=2,
                                  name="pc1")
                    for kp in range(NK // 2):
                        sc0 = ps.tile([P, 2, TOK], f32, tag="sc2", bufs=2,
                                      name="sc0")
                        sc1 = ps.tile([P, 2, TOK], f32, tag="sc2", bufs=2,
                                      name="sc1")
                        for i in range(2):
                            kc = 2 * kp + i
                            r_, j = divmod(kc, 4)
                            nc.tensor.matmul(
                                sc0[:, i, :],
                                ktc[0:DH, r_, 128 * j:128 * (j + 1)],
                                QT[0:DH, c, :], start=True, stop=True)
                            nc.tensor.matmul(
                                sc1[:, i, :],
                                ktc[DH:P, r_, 128 * j:128 * (j + 1)],
                                QT[DH:P, c, :], start=True, stop=True)
                        e0 = sb.tile([P, 2, TOK], bf16, tag="E", bufs=3,
                                     name="e0")
                        nc.scalar.activation(e0[:], sc0[:], AF.Exp,
                                             scale=SCALE)
                        e1 = sb.tile([P, 2, TOK], bf16, tag="E", bufs=3,
                                     name="e1")
                        nc.scalar.activation(e1[:], sc1[:], AF.Exp,
                                             scale=SCALE)
                        for i in range(2):
                            kc = 2 * kp + i
                            r_, j = divmod(kc, 4)
                            nc.tensor.matmul(
                                pc0[:], v_sb[:, r_, j,
                                             HA * 2 * c:HA * 2 * c + HA],
                                e0[:, i, :], start=(kc == 0),
                                stop=(kc == NK - 1))
                            nc.tensor.matmul(
                                pc1[:], v_sb[:, r_, j,
                                             HA * (2 * c + 1):
                                             HA * (2 * c + 1) + HA],
                                e1[:, i, :], start=(kc == 0),
                                stop=(kc == NK - 1))
                    for h, pch in ((0, pc0), (1, pc1)):
                        # softmax denominator r sits on psum partition 64;
                        # spread it across 128 partitions via a DRAM bounce
                        # so the reciprocal runs 128-wide, then bring it back
                        # as a [1, TOK] row for the ones-broadcast matmul.
                        rst = sb.tile([HA, TOK], f32, tag="rst", bufs=2,
                                      name="rst")
                        nc.vector.tensor_copy(out=rst[DH:HA, :],
                                              in_=pch[DH:HA, :])
                        drT = dram.tile([TOK], f32, tag="drT", bufs=4,
                                        name="drT")
                        nc.sync.dma_start(drT[:], rst[DH:HA, :])
                        rT = sb.tile([P, TOK // P], f32, tag="rT", bufs=2,
                                     name="rT")
                        nc.sync.dma_start(
                            rT[:], drT[:].rearrange("(p f) -> p f", p=P))
                        rTr = sb.tile([P, TOK // P], f32r, tag="rT2", bufs=2,
                                      name="rTr")
                        with nc.allow_low_precision("softmax denominators"):
                            nc.vector.reciprocal(rTr[:], rT[:])
                        drT2 = dram.tile([TOK], f32r, tag="drT2", bufs=4,
                                         name="drT2")
                        nc.sync.dma_start(
                            drT2[:].rearrange("(p f) -> p f", p=P), rTr[:])
                        rrow = sb.tile([1, TOK], f32r, tag="rrow", bufs=2,
                                       name="rrow")
                        nc.sync.dma_start(rrow[:], drT2[:].rearrange(
                            "(o t) -> o t", o=1))
                        prr = ps.tile([DH, TOK], f32, tag="mm", bufs=2,
                                      name="prr")
                        nc.tensor.matmul(prr[:], ones[0:1, 0:DH], rrow[:],
                                         start=True, stop=True)
                        rr = sb.tile([DH, TOK], f32, tag="rrep", bufs=2,
                                     name="rr")
                        nc.scalar.copy(rr[:], prr[:])
                        nc.vector.tensor_mul(
                            ctxT[DH * h:DH * (h + 1), c, :],
                            pch[0:DH, :], rr[:])

                # ---------------- output projection + residual -------------
                t1a = sb.tile([P, DC, TOK], f32r, tag="big2", bufs=2,
                              name="t1a")
                for g in range(4):
                    wt = wtile(wp_d, l, g, "wpt")
                    for cc in range(2):
                        c = 2 * g + cc
                        pp = ps.tile([P, TOK], f32, tag="mm", bufs=2,
                                     name="pp")
                        for kc in range(DC):
                            nc.tensor.matmul(
                                pp[:], wt[:, kc, 128 * cc:128 * (cc + 1)],
                                ctxT[:, kc, :],
                                start=(kc == 0), stop=(kc == DC - 1))
                        nc.vector.scalar_tensor_tensor(
                            out=t1a[:, c, :], in0=pp[:],
                            scalar=bp_sb[:, l, c:c + 1],
                            in1=xT[:, c, :].bitcast(f32),
                            op0=OP.add, op1=OP.add)

                xmid = sb.tile([P, DC, TOK], f32r, tag="xT", bufs=2,
                               name="xmid")
                layernorm(l, t1a, g1_sb, be1_sb, xmid)

                # ---------------- FFN --------------------------------------
                t1f = sb.tile([P, DC, TOK], f32r, tag="big2", bufs=2,
                              name="t1f")
                for q in range(4):
                    hT = sb.tile([P, DC, TOK], f32r, tag="hT", bufs=1,
                                 name="hT")
                    for g in range(4):
                        wt = wtile(w1_d, l, 4 * q + g, "w1t")
                        for cc in range(2):
                            fcl = 2 * g + cc
                            ph = ps.tile([P, TOK], f32, tag="mm", bufs=2,
                                         name="ph")
                            for kc in range(DC):
                                nc.tensor.matmul(
                                    ph[:], wt[:, kc, 128 * cc:128 * (cc + 1)],
                                    xmid[:, kc, :],
                                    start=(kc == 0), stop=(kc == DC - 1))
                            fcg = q * DC + fcl
                            nc.scalar.activation(
                                hT[:, fcl, :], ph[:], AF.Relu,
                                bias=b1_sb[:, l, fcg:fcg + 1])
                    for gc in range(4):
                        w2t = wtile(w2_d, l, 4 * q + gc, "w2t")
                        for cc in range(2):
                            c = 2 * gc + cc
                            py = ps.tile([P, TOK], f32, tag="mm", bufs=2,
                                         name="py")
                            for kc in range(DC):
                                nc.tensor.matmul(
                                    py[:],
                                    w2t[:, kc, 128 * cc:128 * (cc + 1)],
                                    hT[:, kc, :],
                                    start=(kc == 0), stop=(kc == DC - 1))
                            if q == 0:
                                nc.vector.scalar_tensor_tensor(
                                    out=t1f[:, c, :], in0=py[:],
                                    scalar=b2_sb[:, l, c:c + 1],
                                    in1=xmid[:, c, :].bitcast(f32),
                                    op0=OP.add, op1=OP.add)
                            else:
                                nc.vector.tensor_add(
                                    t1f[:, c, :], py[:],
                                    t1f[:, c, :].bitcast(f32))

                xnext = sb.tile([P, DC, TOK], f32r, tag="xT", bufs=2,
                                name="xnext")
                layernorm(l, t1f, g2_sb, be2_sb, xnext)
                xT = xnext

            nc.sync.dma_start(
                out_d[:, :].rearrange("(c p) t -> p c t", p=P).bitcast(f32r),
                xT[:])
    nc.finalize()
    return nc


_NC_CACHE = {}


def get_nc(n_layers=L):
    if n_layers not in _NC_CACHE:
        _NC_CACHE[n_layers] = build(n_layers)
    return _NC_CACHE[n_layers]


def _tile_weight(w, G):
    """[L, K, O] -> [L, G_total, P, K//128, 256] matching wtile() blocks.

    For K=D (projections): block g covers out-cols 256g..256g+256, all K.
    For W1/W2 the same formula applies per 1024-col quarter group because
    blocks are indexed 4q+g and cover kc-chunks of the full K dim for W1,
    and kc-local chunks for W2 (handled by the caller's slicing)."""
    Lw, K, O = w.shape
    t = w.reshape(Lw, K // P, P, O // 256, 256).transpose(0, 3, 2, 1, 4)
    return np.ascontiguousarray(t)


def _tile_w2(w2):
    """[L, FF, D] -> [L, 16, P, 8, 256]; block 4q+gc covers W2 rows
    1024q..1024(q+1), cols 256gc..256(gc+1)."""
    Lw = w2.shape[0]
    t = w2.reshape(Lw, 4, 8, P, 4, 256).transpose(0, 1, 4, 3, 2, 5)
    return np.ascontiguousarray(t.reshape(Lw, 16, P, 8, 256))


def run(inputs, n_layers=L, trace=False):
    """inputs: the full setup_inputs() dict. Returns (out, BassKernelResults)."""
    hs = np.asarray(inputs["hidden_states"], np.float32)
    f = lambda k: np.ascontiguousarray(np.asarray(inputs[k], np.float32))
    Wq, Wk, Wv, Wp = f("Wq"), f("Wk"), f("Wv"), f("Wp")
    W1, W2 = f("W1"), f("W2")
    bq, bk, bv, bp = f("bq"), f("bk"), f("bv"), f("bp")
    b1, b2 = f("b1"), f("b2")
    g1, be1, g2, be2 = f("ln1_g"), f("ln1_b"), f("ln2_g"), f("ln2_b")
    # fold the V bias through the output projection: P(V + 1 bv^T) Wp + bp
    # = P V Wp + r*(bv Wp) + bp, and after normalization r/r = 1.
    bp_eff = (bp + np.einsum("ld,ldo->lo", bv, Wp)).astype(np.float32)

    wq_t = _tile_weight(Wq[:n_layers], 4)
    wk_t = _tile_weight(Wk[:n_layers], 4)
    wv_t = _tile_weight(Wv[:n_layers], 4)
    wp_t = _tile_weight(Wp[:n_layers], 4)
    w1_t = _tile_weight(W1[:n_layers], 16)
    w2_t = _tile_w2(W2[:n_layers])

    xflat = hs.reshape(B * S, D)
    in_maps = []
    for i in range(NCORES):
        xTi = np.ascontiguousarray(xflat[i * TOK:(i + 1) * TOK].T)
        in_maps.append(dict(
            xT=xTi,
            wq=wq_t, wk=wk_t, wv=wv_t, wp=wp_t, w1=w1_t, w2=w2_t,
            bq=bq[:n_layers], bk=bk[:n_layers], bp=bp_eff[:n_layers],
            b1=b1[:n_layers], b2=b2[:n_layers], g1=g1[:n_layers],
            be1=be1[:n_layers], g2=g2[:n_layers], be2=be2[:n_layers]))
    nc = get_nc(n_layers)
    res = bass_utils.run_bass_kernel_spmd(
        nc, in_maps, core_ids=list(range(NCORES)), trace=trace)
    out = np.empty((B * S, D), np.float32)
    for i in range(NCORES):
        out[i * TOK:(i + 1) * TOK] = res.results[i]["outT"].T
    return out.reshape(B, S, D), res


def kernel(**inputs):
    out, _ = run(inputs)
    return out


# revision 23
# speedup vs baseline: 1.2900x; 1.0582x over previous
"""4-layer transformer encoder (B=2, S=2048, D=1024, FF=4096, H=16) on 8 TRN2
NeuronCores.

Sharding: 4096 tokens split 512/core (cores 0-3 = batch 0, 4-7 = batch 1).
Weights replicated (host pre-tiles them so every weight DMA is contiguous).
Per layer: local QKV projections in fp32r, AllGather of K^T/V (bf16) within
each 4-core quad, attention computed as E^T = exp(K.Q^T/sqrt(dh)) with the
softmax denominator coming from a ones-augmented V matmul (the ones column is
staged into the gather payload), deferred normalization with one batched
reciprocal per layer, output projection + residual + LayerNorm
(partition-axis stats via ones-matmuls), then FFN + residual + LayerNorm.

Activations live transposed in SBUF (x^T: [D partitions, tokens free]) so no
on-device transposes are needed anywhere; the host transposes the input shard
once and the output shard back.
"""
import sys
if '/opt/trn_rl_repo' not in sys.path:
    sys.path.insert(0, '/opt/trn_rl_repo')

import numpy as np
import ml_dtypes

import concourse.bass as bass
import concourse.mybir as mybir
import concourse.tile as tile
import concourse.bacc as bacc
from concourse import bass_utils

# problem config (hardcoded per contest rules)
L = 4
D = 1024
FF = 4096
H = 16
DH = 64
B = 2
S = 2048
EPS = 1e-6
SCALE = 1.0 / 8.0  # 1/sqrt(DH)

NCORES = 8
TOK = 512           # tokens per core
P = 128
DC = D // P         # 8 d-chunks
FC = FF // P        # 32 ff-chunks
NK = S // P         # 16 k-token chunks
R = 4               # ranks per quad (cores sharing one batch element)
RGROUPS = [[0, 1, 2, 3], [4, 5, 6, 7]]
HA = DH + 1         # V head block augmented with a ones column

dt = mybir.dt
AF = mybir.ActivationFunctionType
OP = mybir.AluOpType


def build(n_layers=L):
    nc = bacc.Bacc("TRN2", target_bir_lowering=False, debug=False,
                   num_devices=NCORES)
    f32, f32r, bf16 = dt.float32, dt.float32r, dt.bfloat16

    xT_d = nc.dram_tensor("xT", [D, TOK], f32, kind="ExternalInput")
    # weights pre-tiled on host: [n_layers, G, P, DC, 256]
    wq_d = nc.dram_tensor("wq", [n_layers, 4, P, DC, 256], bf16,
                          kind="ExternalInput")
    wk_d = nc.dram_tensor("wk", [n_layers, 4, P, DC, 256], bf16,
                          kind="ExternalInput")
    wv_d = nc.dram_tensor("wv", [n_layers, 4, P, DC, 256], bf16,
                          kind="ExternalInput")
    wp_d = nc.dram_tensor("wp", [n_layers, 4, P, DC, 256], bf16,
                          kind="ExternalInput")
    w1_d = nc.dram_tensor("w1", [n_layers, 16, P, DC, 256], bf16,
                          kind="ExternalInput")
    w2_d = nc.dram_tensor("w2", [n_layers, 16, P, DC, 256], bf16,
                          kind="ExternalInput")
    bq_d = nc.dram_tensor("bq", [n_layers, D], f32, kind="ExternalInput")
    bk_d = nc.dram_tensor("bk", [n_layers, D], f32, kind="ExternalInput")
    bp_d = nc.dram_tensor("bp", [n_layers, D], f32, kind="ExternalInput")
    b1_d = nc.dram_tensor("b1", [n_layers, FF], f32, kind="ExternalInput")
    b2_d = nc.dram_tensor("b2", [n_layers, D], f32, kind="ExternalInput")
    g1_d = nc.dram_tensor("g1", [n_layers, D], f32, kind="ExternalInput")
    be1_d = nc.dram_tensor("be1", [n_layers, D], f32, kind="ExternalInput")
    g2_d = nc.dram_tensor("g2", [n_layers, D], f32, kind="ExternalInput")
    be2_d = nc.dram_tensor("be2", [n_layers, D], f32, kind="ExternalInput")
    out_d = nc.dram_tensor("outT", [D, TOK], f32, kind="ExternalOutput")

    with tile.TileContext(nc) as tc:
        with (
            tc.tile_pool(name="pers", bufs=1) as pers,
            tc.tile_pool(name="sb", bufs=1) as sb,
            tc.tile_pool(name="ps", bufs=1, space="PSUM") as ps,
            tc.tile_pool(name="dram", bufs=1, space="DRAM") as dram,
        ):
            ones_f = pers.tile([P, P], f32)
            nc.vector.memset(ones_f[:], 1.0)
            ones = pers.tile([P, P], f32r)
            nc.vector.tensor_copy(out=ones[:], in_=ones_f[:])
            eps_sb = pers.tile([1, 1], f32)
            nc.vector.memset(eps_sb[:], EPS)


            def load_param(name, src, nchunk):
                t = pers.tile([P, n_layers, nchunk], f32, name=name)
                nc.sync.dma_start(
                    t[:], src[:, :].rearrange("l (c p) -> p l c", p=P))
                return t

            bq_sb = load_param("bq_sb", bq_d, DC)
            bk_sb = load_param("bk_sb", bk_d, DC)
            bp_sb = load_param("bp_sb", bp_d, DC)
            b2_sb = load_param("b2_sb", b2_d, DC)
            g1_sb = load_param("g1_sb", g1_d, DC)
            be1_sb = load_param("be1_sb", be1_d, DC)
            g2_sb = load_param("g2_sb", g2_d, DC)
            be2_sb = load_param("be2_sb", be2_d, DC)
            b1_sb = load_param("b1_sb", b1_d, FC)

            xT = sb.tile([P, DC, TOK], f32r, tag="xT", bufs=2, name="xT0")
            nc.sync.dma_start(
                xT[:],
                xT_d[:, :].rearrange("(c p) t -> p c t", p=P).bitcast(f32r))

            def cast_bf16(xsrc, name):
                xb = sb.tile([P, DC, TOK], bf16, tag="xTb", bufs=2, name=name)
                for c in range(DC):
                    nc.vector.tensor_copy(out=xb[:, c, :],
                                          in_=xsrc[:, c, :].bitcast(f32))
                return xb
            xTb = cast_bf16(xT, "xTb0")

            def wtile(w_d, l, g, name):
                t = sb.tile([P, DC, 256], bf16, tag="wblk", bufs=3, name=name)
                nc.sync.dma_start(t[:], w_d[l, g])
                return t

            def layernorm(l, t1, g_sb, be_sb, xout):
                """xout[:, c, :] = LN(t1) over the partition (d) axis."""
                psum_s = ps.tile([1, TOK], f32, tag="mm", bufs=2,
                                 name="psum_s")
                psum_sq = ps.tile([1, TOK], f32, tag="mm", bufs=2,
                                  name="psum_sq")
                for c in range(DC):
                    nc.tensor.matmul(psum_s[:], ones[:, 0:1], t1[:, c, :],
                                     start=(c == 0), stop=(c == DC - 1))
                for c in range(DC):
                    sqc = sb.tile([P, TOK], f32r, tag="sq", bufs=2, name="sqc")
                    nc.scalar.square(sqc[:], t1[:, c, :])
                    nc.tensor.matmul(psum_sq[:], ones[:, 0:1], sqc[:],
                                     start=(c == 0), stop=(c == DC - 1))
                mean = sb.tile([1, TOK], f32r, tag="vec", bufs=4, name="mean")
                nc.vector.tensor_scalar_mul(mean[:], psum_s[:], 1.0 / D)
                ms = sb.tile([1, TOK], f32, tag="vec", bufs=4, name="ms")
                nc.vector.tensor_scalar_mul(ms[:], psum_sq[:], 1.0 / D)
                var = sb.tile([1, TOK], f32, tag="vec", bufs=4, name="var")
                # var = ms - mean*mean = (mean * -mean) * mean + ms
                nc.vector.scalar_tensor_tensor(
                    out=var[:], in0=mean[:].bitcast(f32), scalar=-1.0,
                    in1=mean[:].bitcast(f32), op0=OP.mult, op1=OP.mult)
                nc.vector.tensor_sub(var[:], ms[:], var[:])
                std = sb.tile([1, TOK], f32, tag="vec", bufs=4, name="std")
                nc.scalar.activation(std[:], var[:], AF.Sqrt, bias=eps_sb[:])
                rstd = sb.tile([1, TOK], f32r, tag="vec", bufs=4, name="rstd")
                with nc.allow_low_precision("fp32r rstd for PE broadcast"):
                    nc.vector.reciprocal(rstd[:], std[:])
                pm = ps.tile([P, TOK], f32, tag="mm", bufs=2, name="pm")
                nc.tensor.matmul(pm[:], ones[0:1, :], mean[:],
                                 start=True, stop=True)
                mrep = sb.tile([P, TOK], f32, tag="mrep", bufs=1, name="mrep")
                nc.scalar.copy(mrep[:], pm[:])
                pr = ps.tile([P, TOK], f32, tag="mm", bufs=2, name="pr")
                nc.tensor.matmul(pr[:], ones[0:1, :], rstd[:],
                                 start=True, stop=True)
                rrep = sb.tile([P, TOK], f32, tag="rrepLN", bufs=1,
                               name="rrep")
                nc.scalar.copy(rrep[:], pr[:])
                for c in range(DC):
                    d1 = sb.tile([P, TOK], f32, tag="lnscr", bufs=3,
                                 name="d1")
                    nc.vector.tensor_sub(d1[:], t1[:, c, :].bitcast(f32),
                                         mrep[:])
                    d2 = sb.tile([P, TOK], f32, tag="lnscr", bufs=3,
                                 name="d2")
                    nc.vector.tensor_mul(d2[:], d1[:], rrep[:])
                    nc.vector.tensor_scalar(
                        out=xout[:, c, :], in0=d2[:],
                        scalar1=g_sb[:, l, c:c + 1],
                        scalar2=be_sb[:, l, c:c + 1],
                        op0=OP.mult, op1=OP.add)

            for l in range(n_layers):
                # ---------------- K projection (staged to gather input) ----
                ccK = dram.tile([DC, P, TOK], bf16, tag="ccK", bufs=2,
                                name="ccK")
                for g in range(4):
                    wt = wtile(wk_d, l, g, "wkt")
                    for cc in range(2):
                        c = 2 * g + cc
                        pk = ps.tile([P, TOK], f32, tag="mm", bufs=2,
                                     name="pk")
                        for kc in range(DC):
                            nc.tensor.matmul(
                                pk[:], wt[:, kc, 128 * cc:128 * (cc + 1)],
                                xTb[:, kc, :],
                                start=(kc == 0), stop=(kc == DC - 1))
                        kst = sb.tile([P, TOK], bf16, tag="kvstage", bufs=2,
                                      name="kst")
                        nc.scalar.activation(kst[:], pk[:], AF.Identity,
                                             bias=bk_sb[:, l, c:c + 1])
                        nc.sync.dma_start(ccK[c, :, :], kst[:])
                ccKo = dram.tile([R, DC, P, TOK], bf16, tag="ccKo", bufs=2,
                                 name="ccKo")
                nc.gpsimd.collective_compute(
                    "AllGather", OP.bypass, replica_groups=RGROUPS,
                    ins=[ccK[:].opt()], outs=[ccKo[:].opt()])

                # ---------------- V projection (token-major, augmented) ----
                # ccV[tc, p, h*HA + d] = V[tc*128+p, 64h+d]; col d=64 is 1.0
                ccV = dram.tile([4, P, H * HA], bf16, tag="ccV", bufs=2,
                                name="ccV")
                for g in range(4):
                    wt = wtile(wv_d, l, g, "wvt")
                    for tc in range(4):
                        pv = ps.tile([P, 256], f32, tag="mm", bufs=2,
                                     name="pv")
                        for kc in range(DC):
                            nc.tensor.matmul(
                                pv[:], xTb[:, kc, 128 * tc:128 * (tc + 1)],
                                wt[:, kc, :],
                                start=(kc == 0), stop=(kc == DC - 1))
                        # 256 cols = heads 4g..4g+3; stage with ones column
                        vst = sb.tile([P, 4, HA], bf16, tag="vstage", bufs=2,
                                      name="vst")
                        nc.vector.tensor_copy(
                            out=vst[:, :, 0:DH],
                            in_=pv[:].rearrange("p (h d) -> p h d", d=DH))
                        nc.vector.memset(vst[:, :, DH:HA], 1.0)
                        nc.sync.dma_start(
                            ccV[tc, :, 4 * g * HA:(4 * g + 4) * HA],
                            vst[:].rearrange("p h d -> p (h d)"))
                ccVo = dram.tile([R, 4, P, H * HA], bf16, tag="ccVo", bufs=2,
                                 name="ccVo")
                nc.gpsimd.collective_compute(
                    "AllGather", OP.bypass, replica_groups=RGROUPS,
                    ins=[ccV[:].opt()], outs=[ccVo[:].opt()])

                # ---------------- Q projection (stays local, bf16) ---------
                QT = sb.tile([P, DC, TOK], bf16, tag="QT", bufs=1, name="QT")
                for g in range(4):
                    wt = wtile(wq_d, l, g, "wqt")
                    for cc in range(2):
                        c = 2 * g + cc
                        pq = ps.tile([P, TOK], f32, tag="mm", bufs=2,
                                     name="pq")
                        for kc in range(DC):
                            nc.tensor.matmul(
                                pq[:], wt[:, kc, 128 * cc:128 * (cc + 1)],
                                xTb[:, kc, :],
                                start=(kc == 0), stop=(kc == DC - 1))
                        nc.scalar.activation(QT[:, c, :], pq[:], AF.Identity,
                                             bias=bq_sb[:, l, c:c + 1])

                # ---------------- gathered K/V into SBUF -------------------
                v_sb = sb.tile([P, R, 4, H * HA], bf16, tag="Vg", bufs=1,
                               name="v_sb")
                for r_ in range(R):
                    for tc in range(4):
                        nc.sync.dma_start(v_sb[:, r_, tc, :], ccVo[r_, tc])

                # ---------------- attention, head pair per c ---------------
                # ctx~ (unnormalized) and per-head denominators r; one
                # batched reciprocal at the end of the phase.
                ctxT = sb.tile([P, DC, TOK], bf16, tag="ctxTb", bufs=1,
                               name="ctxT")
                for c in range(DC):
                    ktc = sb.tile([P, R, TOK], bf16, tag="KTc", bufs=2,
                                  name="ktc")
                    nc.sync.dma_start(ktc[:], ccKo[:, c, :, :].rearrange(
                        "r p t -> p r t"))
                    pc0 = ps.tile([HA, TOK], f32, tag="ctx", bufs=2,
                                  name="pc0")
                    pc1 = ps.tile([HA, TOK], f32, tag="ctx", bufs=2,
                                  name="pc1")
                    for kp in range(NK // 2):
                        sc0 = ps.tile([P, 2, TOK], f32, tag="sc2", bufs=2,
                                      name="sc0")
                        sc1 = ps.tile([P, 2, TOK], f32, tag="sc2", bufs=2,
                                      name="sc1")
                        for i in range(2):
                            kc = 2 * kp + i
                            r_, j = divmod(kc, 4)
                            nc.tensor.matmul(
                                sc0[:, i, :],
                                ktc[0:DH, r_, 128 * j:128 * (j + 1)],
                                QT[0:DH, c, :], start=True, stop=True)
                            nc.tensor.matmul(
                                sc1[:, i, :],
                                ktc[DH:P, r_, 128 * j:128 * (j + 1)],
                                QT[DH:P, c, :], start=True, stop=True)
                        e0 = sb.tile([P, 2, TOK], bf16, tag="E", bufs=3,
                                     name="e0")
                        nc.scalar.activation(e0[:], sc0[:], AF.Exp,
                                             scale=SCALE)
                        e1 = sb.tile([P, 2, TOK], bf16, tag="E", bufs=3,
                                     name="e1")
                        nc.scalar.activation(e1[:], sc1[:], AF.Exp,
                                             scale=SCALE)
                        for i in range(2):
                            kc = 2 * kp + i
                            r_, j = divmod(kc, 4)
                            nc.tensor.matmul(
                                pc0[:], v_sb[:, r_, j,
                                             HA * 2 * c:HA * 2 * c + HA],
                                e0[:, i, :], start=(kc == 0),
                                stop=(kc == NK - 1))
                            nc.tensor.matmul(
                                pc1[:], v_sb[:, r_, j,
                                             HA * (2 * c + 1):
                                             HA * (2 * c + 1) + HA],
                                e1[:, i, :], start=(kc == 0),
                                stop=(kc == NK - 1))
                    for h, pch in ((0, pc0), (1, pc1)):
                        # softmax denominator r sits on psum partition 64;
                        # spread it across 128 partitions via a DRAM bounce
                        # so the reciprocal runs 128-wide, then bring it back
                        # as a [1, TOK] row for the ones-broadcast matmul.
                        rst = sb.tile([HA, TOK], f32, tag="rst", bufs=2,
                                      name="rst")
                        nc.vector.tensor_copy(out=rst[DH:HA, :],
                                              in_=pch[DH:HA, :])
                        drT = dram.tile([TOK], f32, tag="drT", bufs=4,
                                        name="drT")
                        nc.sync.dma_start(drT[:], rst[DH:HA, :])
                        rT = sb.tile([P, TOK // P], f32, tag="rT", bufs=2,
                                     name="rT")
                        nc.sync.dma_start(
                            rT[:], drT[:].rearrange("(p f) -> p f", p=P))
                        rTr = sb.tile([P, TOK // P], f32r, tag="rT2", bufs=2,
                                      name="rTr")
                        with nc.allow_low_precision("softmax denominators"):
                            nc.vector.reciprocal(rTr[:], rT[:])
                        drT2 = dram.tile([TOK], f32r, tag="drT2", bufs=4,
                                         name="drT2")
                        nc.sync.dma_start(
                            drT2[:].rearrange("(p f) -> p f", p=P), rTr[:])
                        rrow = sb.tile([1, TOK], f32r, tag="rrow", bufs=2,
                                       name="rrow")
                        nc.sync.dma_start(rrow[:], drT2[:].rearrange(
                            "(o t) -> o t", o=1))
                        prr = ps.tile([DH, TOK], f32, tag="mm", bufs=2,
                                      name="prr")
                        nc.tensor.matmul(prr[:], ones[0:1, 0:DH], rrow[:],
                                         start=True, stop=True)
                        rr = sb.tile([DH, TOK], f32, tag="rrep", bufs=2,
                                     name="rr")
                        nc.scalar.copy(rr[:], prr[:])
                        nc.vector.tensor_mul(
                            ctxT[DH * h:DH * (h + 1), c, :],
                            pch[0:DH, :], rr[:])

                # ---------------- output projection + residual -------------
                t1a = sb.tile([P, DC, TOK], f32r, tag="big2", bufs=1,
                              name="t1a")
                for g in range(4):
                    wt = wtile(wp_d, l, g, "wpt")
                    for cc in range(2):
                        c = 2 * g + cc
                        pp = ps.tile([P, TOK], f32, tag="mm", bufs=2,
                                     name="pp")
                        for kc in range(DC):
                            nc.tensor.matmul(
                                pp[:], wt[:, kc, 128 * cc:128 * (cc + 1)],
                                ctxT[:, kc, :],
                                start=(kc == 0), stop=(kc == DC - 1))
                        nc.vector.scalar_tensor_tensor(
                            out=t1a[:, c, :], in0=pp[:],
                            scalar=bp_sb[:, l, c:c + 1],
                            in1=xT[:, c, :].bitcast(f32),
                            op0=OP.add, op1=OP.add)

                xmid = sb.tile([P, DC, TOK], f32r, tag="xT", bufs=2,
                               name="xmid")
                layernorm(l, t1a, g1_sb, be1_sb, xmid)
                xmidb = cast_bf16(xmid, "xmidb")

                # ---------------- FFN --------------------------------------
                t1f = sb.tile([P, DC, TOK], f32r, tag="big2", bufs=1,
                              name="t1f")
                for q in range(4):
                    hT = sb.tile([P, DC, TOK], bf16, tag="hT", bufs=1,
                                 name="hT")
                    for g in range(4):
                        wt = wtile(w1_d, l, 4 * q + g, "w1t")
                        for cc in range(2):
                            fcl = 2 * g + cc
                            ph = ps.tile([P, TOK], f32, tag="mm", bufs=2,
                                         name="ph")
                            for kc in range(DC):
                                nc.tensor.matmul(
                                    ph[:], wt[:, kc, 128 * cc:128 * (cc + 1)],
                                    xmidb[:, kc, :],
                                    start=(kc == 0), stop=(kc == DC - 1))
                            fcg = q * DC + fcl
                            nc.scalar.activation(
                                hT[:, fcl, :], ph[:], AF.Relu,
                                bias=b1_sb[:, l, fcg:fcg + 1])
                    for gc in range(4):
                        w2t = wtile(w2_d, l, 4 * q + gc, "w2t")
                        for cc in range(2):
                            c = 2 * gc + cc
                            py = ps.tile([P, TOK], f32, tag="mm", bufs=2,
                                         name="py")
                            for kc in range(DC):
                                nc.tensor.matmul(
                                    py[:],
                                    w2t[:, kc, 128 * cc:128 * (cc + 1)],
                                    hT[:, kc, :],
                                    start=(kc == 0), stop=(kc == DC - 1))
                            if q == 0:
                                nc.vector.scalar_tensor_tensor(
                                    out=t1f[:, c, :], in0=py[:],
                                    scalar=b2_sb[:, l, c:c + 1],
                                    in1=xmid[:, c, :].bitcast(f32),
                                    op0=OP.add, op1=OP.add)
                            else:
                                nc.vector.tensor_add(
                                    t1f[:, c, :], py[:],
                                    t1f[:, c, :].bitcast(f32))

                xnext = sb.tile([P, DC, TOK], f32r, tag="xT", bufs=2,
                                name="xnext")
                layernorm(l, t1f, g2_sb, be2_sb, xnext)
                xT = xnext
                xTb = cast_bf16(xT, "xTbn")

            nc.sync.dma_start(
                out_d[:, :].rearrange("(c p) t -> p c t", p=P).bitcast(f32r),
                xT[:])
    nc.finalize()
    return nc


_NC_CACHE = {}


def get_nc(n_layers=L):
    if n_layers not in _NC_CACHE:
        _NC_CACHE[n_layers] = build(n_layers)
    return _NC_CACHE[n_layers]


def _tile_weight(w, G):
    """[L, K, O] -> [L, G_total, P, K//128, 256] matching wtile() blocks.

    For K=D (projections): block g covers out-cols 256g..256g+256, all K.
    For W1/W2 the same formula applies per 1024-col quarter group because
    blocks are indexed 4q+g and cover kc-chunks of the full K dim for W1,
    and kc-local chunks for W2 (handled by the caller's slicing)."""
    Lw, K, O = w.shape
    t = w.reshape(Lw, K // P, P, O // 256, 256).transpose(0, 3, 2, 1, 4)
    return np.ascontiguousarray(t)


def _tile_w2(w2):
    """[L, FF, D] -> [L, 16, P, 8, 256]; block 4q+gc covers W2 rows
    1024q..1024(q+1), cols 256gc..256(gc+1)."""
    Lw = w2.shape[0]
    t = w2.reshape(Lw, 4, 8, P, 4, 256).transpose(0, 1, 4, 3, 2, 5)
    return np.ascontiguousarray(t.reshape(Lw, 16, P, 8, 256))


def run(inputs, n_layers=L, trace=False):
    """inputs: the full setup_inputs() dict. Returns (out, BassKernelResults)."""
    hs = np.asarray(inputs["hidden_states"], np.float32)
    f = lambda k: np.ascontiguousarray(np.asarray(inputs[k], np.float32))
    Wq, Wk, Wv, Wp = f("Wq"), f("Wk"), f("Wv"), f("Wp")
    W1, W2 = f("W1"), f("W2")
    bq, bk, bv, bp = f("bq"), f("bk"), f("bv"), f("bp")
    b1, b2 = f("b1"), f("b2")
    g1, be1, g2, be2 = f("ln1_g"), f("ln1_b"), f("ln2_g"), f("ln2_b")
    # fold the V bias through the output projection: P(V + 1 bv^T) Wp + bp
    # = P V Wp + r*(bv Wp) + bp, and after normalization r/r = 1.
    bp_eff = (bp + np.einsum("ld,ldo->lo", bv, Wp)).astype(np.float32)

    bf = ml_dtypes.bfloat16
    wq_t = _tile_weight(Wq[:n_layers], 4).astype(bf)
    wk_t = _tile_weight(Wk[:n_layers], 4).astype(bf)
    wv_t = _tile_weight(Wv[:n_layers], 4).astype(bf)
    wp_t = _tile_weight(Wp[:n_layers], 4).astype(bf)
    w1_t = _tile_weight(W1[:n_layers], 16).astype(bf)
    w2_t = _tile_w2(W2[:n_layers]).astype(bf)

    xflat = hs.reshape(B * S, D)
    in_maps = []
    for i in range(NCORES):
        xTi = np.ascontiguousarray(xflat[i * TOK:(i + 1) * TOK].T)
        in_maps.append(dict(
            xT=xTi,
            wq=wq_t, wk=wk_t, wv=wv_t, wp=wp_t, w1=w1_t, w2=w2_t,
            bq=bq[:n_layers], bk=bk[:n_layers], bp=bp_eff[:n_layers],
            b1=b1[:n_layers], b2=b2[:n_layers], g1=g1[:n_layers],
            be1=be1[:n_layers], g2=g2[:n_layers], be2=be2[:n_layers]))
    nc = get_nc(n_layers)
    res = bass_utils.run_bass_kernel_spmd(
        nc, in_maps, core_ids=list(range(NCORES)), trace=trace)
    out = np.empty((B * S, D), np.float32)
    for i in range(NCORES):
        out[i * TOK:(i + 1) * TOK] = res.results[i]["outT"].T
    return out.reshape(B, S, D), res


def kernel(**inputs):
    out, _ = run(inputs)
    return out


# revision 24
# speedup vs baseline: 1.3015x; 1.0090x over previous
"""4-layer transformer encoder (B=2, S=2048, D=1024, FF=4096, H=16) on 8 TRN2
NeuronCores.

Sharding: 4096 tokens split 512/core (cores 0-3 = batch 0, 4-7 = batch 1).
Weights replicated (host pre-tiles them so every weight DMA is contiguous).
Per layer: local QKV projections in fp32r, AllGather of K^T/V (bf16) within
each 4-core quad, attention computed as E^T = exp(K.Q^T/sqrt(dh)) with the
softmax denominator coming from a ones-augmented V matmul (the ones column is
staged into the gather payload), deferred normalization with one batched
reciprocal per layer, output projection + residual + LayerNorm
(partition-axis stats via ones-matmuls), then FFN + residual + LayerNorm.

Activations live transposed in SBUF (x^T: [D partitions, tokens free]) so no
on-device transposes are needed anywhere; the host transposes the input shard
once and the output shard back.
"""
import sys
if '/opt/trn_rl_repo' not in sys.path:
    sys.path.insert(0, '/opt/trn_rl_repo')

import numpy as np
import ml_dtypes

import concourse.bass as bass
import concourse.mybir as mybir
import concourse.tile as tile
import concourse.bacc as bacc
from concourse import bass_utils

# problem config (hardcoded per contest rules)
L = 4
D = 1024
FF = 4096
H = 16
DH = 64
B = 2
S = 2048
EPS = 1e-6
SCALE = 1.0 / 8.0  # 1/sqrt(DH)

NCORES = 8
TOK = 512           # tokens per core
P = 128
DC = D // P         # 8 d-chunks
FC = FF // P        # 32 ff-chunks
NK = S // P         # 16 k-token chunks
R = 4               # ranks per quad (cores sharing one batch element)
RGROUPS = [[0, 1, 2, 3], [4, 5, 6, 7]]
HA = DH + 1         # V head block augmented with a ones column

dt = mybir.dt
AF = mybir.ActivationFunctionType
OP = mybir.AluOpType


def build(n_layers=L):
    nc = bacc.Bacc("TRN2", target_bir_lowering=False, debug=False,
                   num_devices=NCORES)
    f32, f32r, bf16 = dt.float32, dt.float32r, dt.bfloat16

    xT_d = nc.dram_tensor("xT", [D, TOK], f32, kind="ExternalInput")
    # weights pre-tiled on host: [n_layers, G, P, DC, 256]
    wq_d = nc.dram_tensor("wq", [n_layers, 4, P, DC, 256], bf16,
                          kind="ExternalInput")
    wk_d = nc.dram_tensor("wk", [n_layers, 4, P, DC, 256], bf16,
                          kind="ExternalInput")
    wv_d = nc.dram_tensor("wv", [n_layers, 4, P, DC, 256], bf16,
                          kind="ExternalInput")
    wp_d = nc.dram_tensor("wp", [n_layers, 4, P, DC, 256], bf16,
                          kind="ExternalInput")
    w1_d = nc.dram_tensor("w1", [n_layers, 16, P, DC, 256], bf16,
                          kind="ExternalInput")
    w2_d = nc.dram_tensor("w2", [n_layers, 16, P, DC, 256], bf16,
                          kind="ExternalInput")
    bq_d = nc.dram_tensor("bq", [n_layers, D], f32, kind="ExternalInput")
    bk_d = nc.dram_tensor("bk", [n_layers, D], f32, kind="ExternalInput")
    bp_d = nc.dram_tensor("bp", [n_layers, D], f32, kind="ExternalInput")
    b1_d = nc.dram_tensor("b1", [n_layers, FF], f32, kind="ExternalInput")
    b2_d = nc.dram_tensor("b2", [n_layers, D], f32, kind="ExternalInput")
    g1_d = nc.dram_tensor("g1", [n_layers, D], f32, kind="ExternalInput")
    be1_d = nc.dram_tensor("be1", [n_layers, D], f32, kind="ExternalInput")
    g2_d = nc.dram_tensor("g2", [n_layers, D], f32, kind="ExternalInput")
    be2_d = nc.dram_tensor("be2", [n_layers, D], f32, kind="ExternalInput")
    out_d = nc.dram_tensor("outT", [D, TOK], f32, kind="ExternalOutput")

    with tile.TileContext(nc) as tc:
        with (
            tc.tile_pool(name="pers", bufs=1) as pers,
            tc.tile_pool(name="sb", bufs=1) as sb,
            tc.tile_pool(name="ps", bufs=1, space="PSUM") as ps,
            tc.tile_pool(name="dram", bufs=1, space="DRAM") as dram,
        ):
            ones_f = pers.tile([P, P], f32)
            nc.vector.memset(ones_f[:], 1.0)
            ones = pers.tile([P, P], f32r)
            nc.vector.tensor_copy(out=ones[:], in_=ones_f[:])
            eps_sb = pers.tile([1, 1], f32)
            nc.vector.memset(eps_sb[:], EPS)


            def load_param(name, src, nchunk):
                t = pers.tile([P, n_layers, nchunk], f32, name=name)
                nc.sync.dma_start(
                    t[:], src[:, :].rearrange("l (c p) -> p l c", p=P))
                return t

            bq_sb = load_param("bq_sb", bq_d, DC)
            bk_sb = load_param("bk_sb", bk_d, DC)
            bp_sb = load_param("bp_sb", bp_d, DC)
            b2_sb = load_param("b2_sb", b2_d, DC)
            g1_sb = load_param("g1_sb", g1_d, DC)
            be1_sb = load_param("be1_sb", be1_d, DC)
            g2_sb = load_param("g2_sb", g2_d, DC)
            be2_sb = load_param("be2_sb", be2_d, DC)
            b1_sb = load_param("b1_sb", b1_d, FC)

            xT = sb.tile([P, DC, TOK], f32r, tag="xT", bufs=2, name="xT0")
            nc.sync.dma_start(
                xT[:],
                xT_d[:, :].rearrange("(c p) t -> p c t", p=P).bitcast(f32r))

            def cast_bf16(xsrc, name):
                xb = sb.tile([P, DC, TOK], bf16, tag="xTb", bufs=2, name=name)
                for c in range(DC):
                    nc.vector.tensor_copy(out=xb[:, c, :],
                                          in_=xsrc[:, c, :].bitcast(f32))
                return xb
            xTb = cast_bf16(xT, "xTb0")

            def wtile(w_d, l, g, name):
                t = sb.tile([P, DC, 256], bf16, tag="wblk", bufs=3, name=name)
                nc.sync.dma_start(t[:], w_d[l, g])
                return t

            def layernorm(l, t1, g_sb, be_sb, xout):
                """xout[:, c, :] = LN(t1) over the partition (d) axis."""
                psum_s = ps.tile([1, TOK], f32, tag="mm", bufs=2,
                                 name="psum_s")
                psum_sq = ps.tile([1, TOK], f32, tag="mm", bufs=2,
                                  name="psum_sq")
                for c in range(DC):
                    nc.tensor.matmul(psum_s[:], ones[:, 0:1], t1[:, c, :],
                                     start=(c == 0), stop=(c == DC - 1))
                for c in range(DC):
                    sqc = sb.tile([P, TOK], f32r, tag="sq", bufs=2, name="sqc")
                    nc.scalar.square(sqc[:], t1[:, c, :])
                    nc.tensor.matmul(psum_sq[:], ones[:, 0:1], sqc[:],
                                     start=(c == 0), stop=(c == DC - 1))
                mean = sb.tile([1, TOK], f32r, tag="vec", bufs=4, name="mean")
                nc.vector.tensor_scalar_mul(mean[:], psum_s[:], 1.0 / D)
                ms = sb.tile([1, TOK], f32, tag="vec", bufs=4, name="ms")
                nc.vector.tensor_scalar_mul(ms[:], psum_sq[:], 1.0 / D)
                var = sb.tile([1, TOK], f32, tag="vec", bufs=4, name="var")
                # var = ms - mean*mean = (mean * -mean) * mean + ms
                nc.vector.scalar_tensor_tensor(
                    out=var[:], in0=mean[:].bitcast(f32), scalar=-1.0,
                    in1=mean[:].bitcast(f32), op0=OP.mult, op1=OP.mult)
                nc.vector.tensor_sub(var[:], ms[:], var[:])
                std = sb.tile([1, TOK], f32, tag="vec", bufs=4, name="std")
                nc.scalar.activation(std[:], var[:], AF.Sqrt, bias=eps_sb[:])
                rstd = sb.tile([1, TOK], f32r, tag="vec", bufs=4, name="rstd")
                with nc.allow_low_precision("fp32r rstd for PE broadcast"):
                    nc.vector.reciprocal(rstd[:], std[:])
                pm = ps.tile([P, TOK], f32, tag="mm", bufs=2, name="pm")
                nc.tensor.matmul(pm[:], ones[0:1, :], mean[:],
                                 start=True, stop=True)
                mrep = sb.tile([P, TOK], f32, tag="mrep", bufs=1, name="mrep")
                nc.scalar.copy(mrep[:], pm[:])
                pr = ps.tile([P, TOK], f32, tag="mm", bufs=2, name="pr")
                nc.tensor.matmul(pr[:], ones[0:1, :], rstd[:],
                                 start=True, stop=True)
                rrep = sb.tile([P, TOK], f32, tag="rrepLN", bufs=1,
                               name="rrep")
                nc.scalar.copy(rrep[:], pr[:])
                for c in range(DC):
                    d1 = sb.tile([P, TOK], f32, tag="lnscr", bufs=3,
                                 name="d1")
                    nc.vector.tensor_sub(d1[:], t1[:, c, :].bitcast(f32),
                                         mrep[:])
                    d2 = sb.tile([P, TOK], f32, tag="lnscr", bufs=3,
                                 name="d2")
                    nc.vector.tensor_mul(d2[:], d1[:], rrep[:])
                    nc.vector.tensor_scalar(
                        out=xout[:, c, :], in0=d2[:],
                        scalar1=g_sb[:, l, c:c + 1],
                        scalar2=be_sb[:, l, c:c + 1],
                        op0=OP.mult, op1=OP.add)

            for l in range(n_layers):
                # ---------------- K projection (staged to gather input) ----
                ccK = dram.tile([DC, P, TOK], bf16, tag="ccK", bufs=2,
                                name="ccK")
                for g in range(4):
                    wt = wtile(wk_d, l, g, "wkt")
                    for cc in range(2):
                        c = 2 * g + cc
                        pk = ps.tile([P, TOK], f32, tag="mm", bufs=2,
                                     name="pk")
                        for kc in range(DC):
                            nc.tensor.matmul(
                                pk[:], wt[:, kc, 128 * cc:128 * (cc + 1)],
                                xTb[:, kc, :],
                                start=(kc == 0), stop=(kc == DC - 1))
                        kst = sb.tile([P, TOK], bf16, tag="kvstage", bufs=2,
                                      name="kst")
                        nc.scalar.activation(kst[:], pk[:], AF.Identity,
                                             bias=bk_sb[:, l, c:c + 1])
                        nc.sync.dma_start(ccK[c, :, :], kst[:])
                ccKo = dram.tile([R, DC, P, TOK], bf16, tag="ccKo", bufs=2,
                                 name="ccKo")
                nc.gpsimd.collective_compute(
                    "AllGather", OP.bypass, replica_groups=RGROUPS,
                    ins=[ccK[:].opt()], outs=[ccKo[:].opt()])

                # ---------------- V projection (token-major, augmented) ----
                # ccV[tc, p, h*HA + d] = V[tc*128+p, 64h+d]; col d=64 is 1.0
                ccV = dram.tile([4, P, H * HA], bf16, tag="ccV", bufs=2,
                                name="ccV")
                for g in range(4):
                    wt = wtile(wv_d, l, g, "wvt")
                    for tc in range(4):
                        pv = ps.tile([P, 256], f32, tag="mm", bufs=2,
                                     name="pv")
                        for kc in range(DC):
                            nc.tensor.matmul(
                                pv[:], xTb[:, kc, 128 * tc:128 * (tc + 1)],
                                wt[:, kc, :],
                                start=(kc == 0), stop=(kc == DC - 1))
                        # 256 cols = heads 4g..4g+3; stage with ones column
                        vst = sb.tile([P, 4, HA], bf16, tag="vstage", bufs=2,
                                      name="vst")
                        nc.vector.tensor_copy(
                            out=vst[:, :, 0:DH],
                            in_=pv[:].rearrange("p (h d) -> p h d", d=DH))
                        nc.vector.memset(vst[:, :, DH:HA], 1.0)
                        nc.sync.dma_start(
                            ccV[tc, :, 4 * g * HA:(4 * g + 4) * HA],
                            vst[:].rearrange("p h d -> p (h d)"))
                ccVo = dram.tile([R, 4, P, H * HA], bf16, tag="ccVo", bufs=2,
                                 name="ccVo")
                nc.gpsimd.collective_compute(
                    "AllGather", OP.bypass, replica_groups=RGROUPS,
                    ins=[ccV[:].opt()], outs=[ccVo[:].opt()])

                # ---------------- Q projection (stays local, bf16) ---------
                QT = sb.tile([P, DC, TOK], bf16, tag="QT", bufs=1, name="QT")
                for g in range(4):
                    wt = wtile(wq_d, l, g, "wqt")
                    for cc in range(2):
                        c = 2 * g + cc
                        pq = ps.tile([P, TOK], f32, tag="mm", bufs=2,
                                     name="pq")
                        for kc in range(DC):
                            nc.tensor.matmul(
                                pq[:], wt[:, kc, 128 * cc:128 * (cc + 1)],
                                xTb[:, kc, :],
                                start=(kc == 0), stop=(kc == DC - 1))
                        nc.scalar.activation(QT[:, c, :], pq[:], AF.Identity,
                                             bias=bq_sb[:, l, c:c + 1])

                # ---------------- gathered K/V into SBUF -------------------
                v_sb = sb.tile([P, R, 4, H * HA], bf16, tag="Vg", bufs=1,
                               name="v_sb")
                for r_ in range(R):
                    for tc in range(4):
                        nc.sync.dma_start(v_sb[:, r_, tc, :], ccVo[r_, tc])

                # ---------------- attention, head pair per c ---------------
                # ctx~ (unnormalized) and per-head denominators r; one
                # batched reciprocal at the end of the phase.
                ctxT = sb.tile([P, DC, TOK], bf16, tag="ctxTb", bufs=1,
                               name="ctxT")
                for c in range(DC):
                    ktc = sb.tile([P, R, TOK], bf16, tag="KTc", bufs=2,
                                  name="ktc")
                    nc.sync.dma_start(ktc[:], ccKo[:, c, :, :].rearrange(
                        "r p t -> p r t"))
                    pc0 = ps.tile([HA, TOK], f32, tag="ctx", bufs=2,
                                  name="pc0")
                    pc1 = ps.tile([HA, TOK], f32, tag="ctx", bufs=2,
                                  name="pc1")
                    for kp in range(NK // 2):
                        sc0 = ps.tile([P, 2, TOK], f32, tag="sc2", bufs=2,
                                      name="sc0")
                        sc1 = ps.tile([P, 2, TOK], f32, tag="sc2", bufs=2,
                                      name="sc1")
                        for i in range(2):
                            kc = 2 * kp + i
                            r_, j = divmod(kc, 4)
                            nc.tensor.matmul(
                                sc0[:, i, :],
                                ktc[0:DH, r_, 128 * j:128 * (j + 1)],
                                QT[0:DH, c, :], start=True, stop=True,
                                tile_position=(0, 0))
                            nc.tensor.matmul(
                                sc1[:, i, :],
                                ktc[DH:P, r_, 128 * j:128 * (j + 1)],
                                QT[DH:P, c, :], start=True, stop=True,
                                tile_position=(64, 0))
                        e0 = sb.tile([P, 2, TOK], bf16, tag="E", bufs=4,
                                     name="e0")
                        nc.scalar.activation(e0[:], sc0[:], AF.Exp,
                                             scale=SCALE)
                        e1 = sb.tile([P, 2, TOK], bf16, tag="E", bufs=4,
                                     name="e1")
                        nc.scalar.activation(e1[:], sc1[:], AF.Exp,
                                             scale=SCALE)
                        for i in range(2):
                            kc = 2 * kp + i
                            r_, j = divmod(kc, 4)
                            nc.tensor.matmul(
                                pc0[:], v_sb[:, r_, j,
                                             HA * 2 * c:HA * 2 * c + HA],
                                e0[:, i, :], start=(kc == 0),
                                stop=(kc == NK - 1))
                            nc.tensor.matmul(
                                pc1[:], v_sb[:, r_, j,
                                             HA * (2 * c + 1):
                                             HA * (2 * c + 1) + HA],
                                e1[:, i, :], start=(kc == 0),
                                stop=(kc == NK - 1))
                    for h, pch in ((0, pc0), (1, pc1)):
                        # softmax denominator r sits on psum partition 64;
                        # spread it across 128 partitions via a DRAM bounce
                        # so the reciprocal runs 128-wide, then bring it back
                        # as a [1, TOK] row for the ones-broadcast matmul.
                        rst = sb.tile([HA, TOK], f32, tag="rst", bufs=2,
                                      name="rst")
                        nc.vector.tensor_copy(out=rst[DH:HA, :],
                                              in_=pch[DH:HA, :])
                        drT = dram.tile([TOK], f32, tag="drT", bufs=4,
                                        name="drT")
                        nc.sync.dma_start(drT[:], rst[DH:HA, :])
                        rT = sb.tile([P, TOK // P], f32, tag="rT", bufs=2,
                                     name="rT")
                        nc.sync.dma_start(
                            rT[:], drT[:].rearrange("(p f) -> p f", p=P))
                        rTr = sb.tile([P, TOK // P], f32r, tag="rT2", bufs=2,
                                      name="rTr")
                        with nc.allow_low_precision("softmax denominators"):
                            nc.vector.reciprocal(rTr[:], rT[:])
                        drT2 = dram.tile([TOK], f32r, tag="drT2", bufs=4,
                                         name="drT2")
                        nc.sync.dma_start(
                            drT2[:].rearrange("(p f) -> p f", p=P), rTr[:])
                        rrow = sb.tile([1, TOK], f32r, tag="rrow", bufs=2,
                                       name="rrow")
                        nc.sync.dma_start(rrow[:], drT2[:].rearrange(
                            "(o t) -> o t", o=1))
                        prr = ps.tile([DH, TOK], f32, tag="mm", bufs=2,
                                      name="prr")
                        nc.tensor.matmul(prr[:], ones[0:1, 0:DH], rrow[:],
                                         start=True, stop=True)
                        rr = sb.tile([DH, TOK], f32, tag="rrep", bufs=2,
                                     name="rr")
                        nc.scalar.copy(rr[:], prr[:])
                        nc.vector.tensor_mul(
                            ctxT[DH * h:DH * (h + 1), c, :],
                            pch[0:DH, :], rr[:])

                # ---------------- output projection + residual -------------
                t1a = sb.tile([P, DC, TOK], f32r, tag="big2", bufs=1,
                              name="t1a")
                for g in range(4):
                    wt = wtile(wp_d, l, g, "wpt")
                    for cc in range(2):
                        c = 2 * g + cc
                        pp = ps.tile([P, TOK], f32, tag="mm", bufs=2,
                                     name="pp")
                        for kc in range(DC):
                            nc.tensor.matmul(
                                pp[:], wt[:, kc, 128 * cc:128 * (cc + 1)],
                                ctxT[:, kc, :],
                                start=(kc == 0), stop=(kc == DC - 1))
                        nc.vector.scalar_tensor_tensor(
                            out=t1a[:, c, :], in0=pp[:],
                            scalar=bp_sb[:, l, c:c + 1],
                            in1=xT[:, c, :].bitcast(f32),
                            op0=OP.add, op1=OP.add)

                xmid = sb.tile([P, DC, TOK], f32r, tag="xT", bufs=2,
                               name="xmid")
                layernorm(l, t1a, g1_sb, be1_sb, xmid)
                xmidb = cast_bf16(xmid, "xmidb")

                # ---------------- FFN --------------------------------------
                t1f = sb.tile([P, DC, TOK], f32r, tag="big2", bufs=1,
                              name="t1f")
                for q in range(4):
                    hT = sb.tile([P, DC, TOK], bf16, tag="hT", bufs=1,
                                 name="hT")
                    for g in range(4):
                        wt = wtile(w1_d, l, 4 * q + g, "w1t")
                        for cc in range(2):
                            fcl = 2 * g + cc
                            ph = ps.tile([P, TOK], f32, tag="mm", bufs=2,
                                         name="ph")
                            for kc in range(DC):
                                nc.tensor.matmul(
                                    ph[:], wt[:, kc, 128 * cc:128 * (cc + 1)],
                                    xmidb[:, kc, :],
                                    start=(kc == 0), stop=(kc == DC - 1))
                            fcg = q * DC + fcl
                            nc.scalar.activation(
                                hT[:, fcl, :], ph[:], AF.Relu,
                                bias=b1_sb[:, l, fcg:fcg + 1])
                    for gc in range(4):
                        w2t = wtile(w2_d, l, 4 * q + gc, "w2t")
                        for cc in range(2):
                            c = 2 * gc + cc
                            py = ps.tile([P, TOK], f32, tag="mm", bufs=2,
                                         name="py")
                            for kc in range(DC):
                                nc.tensor.matmul(
                                    py[:],
                                    w2t[:, kc, 128 * cc:128 * (cc + 1)],
                                    hT[:, kc, :],
                                    start=(kc == 0), stop=(kc == DC - 1))
                            if q == 0:
                                nc.vector.scalar_tensor_tensor(
                                    out=t1f[:, c, :], in0=py[:],
                                    scalar=b2_sb[:, l, c:c + 1],
                                    in1=xmid[:, c, :].bitcast(f32),
                                    op0=OP.add, op1=OP.add)
                            else:
                                nc.vector.tensor_add(
                                    t1f[:, c, :], py[:],
                                    t1f[:, c, :].bitcast(f32))

                xnext = sb.tile([P, DC, TOK], f32r, tag="xT", bufs=2,
                                name="xnext")
                layernorm(l, t1f, g2_sb, be2_sb, xnext)
                xT = xnext
                xTb = cast_bf16(xT, "xTbn")

            nc.sync.dma_start(
                out_d[:, :].rearrange("(c p) t -> p c t", p=P).bitcast(f32r),
                xT[:])
    nc.finalize()
    return nc


_NC_CACHE = {}


def get_nc(n_layers=L):
    if n_layers not in _NC_CACHE:
        _NC_CACHE[n_layers] = build(n_layers)
    return _NC_CACHE[n_layers]


def _tile_weight(w, G):
    """[L, K, O] -> [L, G_total, P, K//128, 256] matching wtile() blocks.

    For K=D (projections): block g covers out-cols 256g..256g+256, all K.
    For W1/W2 the same formula applies per 1024-col quarter group because
    blocks are indexed 4q+g and cover kc-chunks of the full K dim for W1,
    and kc-local chunks for W2 (handled by the caller's slicing)."""
    Lw, K, O = w.shape
    t = w.reshape(Lw, K // P, P, O // 256, 256).transpose(0, 3, 2, 1, 4)
    return np.ascontiguousarray(t)


def _tile_w2(w2):
    """[L, FF, D] -> [L, 16, P, 8, 256]; block 4q+gc covers W2 rows
    1024q..1024(q+1), cols 256gc..256(gc+1)."""
    Lw = w2.shape[0]
    t = w2.reshape(Lw, 4, 8, P, 4, 256).transpose(0, 1, 4, 3, 2, 5)
    return np.ascontiguousarray(t.reshape(Lw, 16, P, 8, 256))


def run(inputs, n_layers=L, trace=False):
    """inputs: the full setup_inputs() dict. Returns (out, BassKernelResults)."""
    hs = np.asarray(inputs["hidden_states"], np.float32)
    f = lambda k: np.ascontiguousarray(np.asarray(inputs[k], np.float32))
    Wq, Wk, Wv, Wp = f("Wq"), f("Wk"), f("Wv"), f("Wp")
    W1, W2 = f("W1"), f("W2")
    bq, bk, bv, bp = f("bq"), f("bk"), f("bv"), f("bp")
    b1, b2 = f("b1"), f("b2")
    g1, be1, g2, be2 = f("ln1_g"), f("ln1_b"), f("ln2_g"), f("ln2_b")
    # fold the V bias through the output projection: P(V + 1 bv^T) Wp + bp
    # = P V Wp + r*(bv Wp) + bp, and after normalization r/r = 1.
    bp_eff = (bp + np.einsum("ld,ldo->lo", bv, Wp)).astype(np.float32)

    bf = ml_dtypes.bfloat16
    wq_t = _tile_weight(Wq[:n_layers], 4).astype(bf)
    wk_t = _tile_weight(Wk[:n_layers], 4).astype(bf)
    wv_t = _tile_weight(Wv[:n_layers], 4).astype(bf)
    wp_t = _tile_weight(Wp[:n_layers], 4).astype(bf)
    w1_t = _tile_weight(W1[:n_layers], 16).astype(bf)
    w2_t = _tile_w2(W2[:n_layers]).astype(bf)

    xflat = hs.reshape(B * S, D)
    in_maps = []
    for i in range(NCORES):
        xTi = np.ascontiguousarray(xflat[i * TOK:(i + 1) * TOK].T)
        in_maps.append(dict(
            xT=xTi,
            wq=wq_t, wk=wk_t, wv=wv_t, wp=wp_t, w1=w1_t, w2=w2_t,
            bq=bq[:n_layers], bk=bk[:n_layers], bp=bp_eff[:n_layers],
            b1=b1[:n_layers], b2=b2[:n_layers], g1=g1[:n_layers],
            be1=be1[:n_layers], g2=g2[:n_layers], be2=be2[:n_layers]))
    nc = get_nc(n_layers)
    res = bass_utils.run_bass_kernel_spmd(
        nc, in_maps, core_ids=list(range(NCORES)), trace=trace)
    out = np.empty((B * S, D), np.float32)
    for i in range(NCORES):
        out[i * TOK:(i + 1) * TOK] = res.results[i]["outT"].T
    return out.reshape(B, S, D), res


def kernel(**inputs):
    out, _ = run(inputs)
    return out
